# revision 79
# baseline (speedup 1.0000x reference)
"""Trainium2 Bass kernel for nn_ConformerEncoderLayer (B=16, L=512, D=512, H=8, FF=2048).

Sharding: data-parallel over batch across 8 NeuronCores (2 batch elems / core).
Device layout: feature-major residual stream x_fm [D=512 (4x128 chunks), T=1024].

v4 (fp8): both FFN matmul pairs and the depthwise conv run fp8e4m3 with
DoubleRow (two K-tiles per instruction = 2x PE throughput; weights host-scaled
x1024 into the fp8 normal range, descaled via the ACT free affine / DVE
immediates). The conv input is duplicated into an even + one-column-shifted
odd copy so DoubleRow tap PAIRS read a standard strided [128, 2, 512] AP; the
per-channel fp8 scaling of the conv taps is absorbed exactly by BatchNorm.
The q/k projections also run fp8 DoubleRow (rope output hr stored fp8); GLU /
pointwise / scores / AV / out-proj stay bf16 (error budget: measured rel err
1.60e-2 vs the 2e-2 gate; numpy bit-accurate sim attributes per-path fp8 error).
Rope tables are bf16 (DVE 2x packed mode, half the DMA). BatchNorm stats use
one batched 8-core all-reduce with a PE junk-matmul keep-warm bridging its
latency; pointwise partials and per-chunk normalize pipeline into the tail.
Constant DMAs are queued before the 2MB x transfer so the PE warmup burst
(HAM un-throttle) is not blocked; E->O conv copies ride the gpsimd queue to
avoid head-of-line blocking the sync queue's weight prefetches.

v3: token-major LayerNorm stats (PE transpose + 128-lane math + stride-0
matmul broadcast); softmax normalization deferred to batch end (one ACT
reciprocal per batch, no ACT-table thrash in the head loop); weight pools
hoisted so FFN2/conv weight DMAs prefetch during earlier phases; PE warmup
burst for the HAM clock.
"""
import sys

sys.path.insert(0, "/opt/trn_rl_repo")

import contextlib
import numpy as np
import ml_dtypes
import concourse.bacc as bacc
import concourse.tile as tile
from concourse import mybir
from concourse.bass_utils import run_bass_kernel_spmd

FP32 = mybir.dt.float32
FP32R = mybir.dt.float32r
BF16 = mybir.dt.bfloat16
FP8 = mybir.dt.float8e4
NP8 = ml_dtypes.float8_e4m3
SCL = 1024.0          # host-side weight pre-scale (2^10) for fp8 range
ISCL = 1.0 / SCL
KTAP2 = 32            # taps padded to even count for DoubleRow pairs

N_CORES = 8
B, L, D, H, FF = 16, 512, 512, 8, 2048
HD = D // H                # 64
BL = B // N_CORES          # 2 batch elems per core
T = BL * L                 # 1024 tokens per core
NC = D // 128              # 4 feature chunks
ROPE_BASE = 10000.0
LN_EPS = 1e-5
BN_EPS = 1e-5
KTAP = 31
PAD = 15
CONVW = PAD + L + PAD + L + PAD          # zeros|b0|zeros|b1|zeros = 1069
OFF_B = (PAD, PAD + L + PAD)             # start col of each batch's data

_CACHE = {}


# ---------------------------------------------------------------- host prep

def _prep_host(inp):
    f32 = np.float32
    x = np.asarray(inp["x"], f32)
    ln = {k: np.asarray(inp[k], f32) for k in
          ("ln1_g", "ln1_b", "ln2_g", "ln2_b", "ln3_g", "ln3_b",
           "ln4_g", "ln4_b", "ln5_g", "ln5_b")}

    def colvec(b, n):  # [F] -> [128, F//128]  (tile[p, i] = b[i*128+p])
        return np.ascontiguousarray(np.asarray(b, f32).reshape(n, 128).T)

    w_ff1a = np.asarray(inp["w_ff1a"], f32)
    w_ff1b = np.asarray(inp["w_ff1b"], f32)
    w_ff2a = np.asarray(inp["w_ff2a"], f32)
    w_ff2b = np.asarray(inp["w_ff2b"], f32)

    def to8(w):  # pre-scaled fp8 weight (descaled on device via ISCL)
        return np.ascontiguousarray(np.clip(w * SCL, -224, 224).astype(NP8))

    d = {}
    ba1 = np.asarray(inp["b_ff1a"], f32) + ln["ln1_b"] @ w_ff1a.T
    ba2 = np.asarray(inp["b_ff2a"], f32) + ln["ln4_b"] @ w_ff2a.T
    d["wff1a"] = to8((w_ff1a * ln["ln1_g"][None, :]).T)
    d["bff1a"] = colvec(ba1, 16)
    d["wff1b"] = to8(0.5 * w_ff1b.T)
    d["bff1b"] = colvec(0.5 * np.asarray(inp["b_ff1b"], f32), 4)
    d["wff2a"] = to8((w_ff2a * ln["ln4_g"][None, :]).T)
    d["bff2a"] = colvec(ba2, 16)
    d["wff2b"] = to8(0.5 * w_ff2b.T)
    d["bff2b"] = colvec(0.5 * np.asarray(inp["b_ff2b"], f32), 4)
    has_bffa = bool(np.any(ba1) or np.any(ba2))
    has_bffb = bool(np.any(inp["b_ff1b"]) or np.any(inp["b_ff2b"]))

    w_in = np.asarray(inp["w_in"], f32)
    b_in = np.asarray(inp["b_in"], f32)
    wq, wk, wv = w_in[:D], w_in[D:2 * D], w_in[2 * D:]
    bq, bk, bv = b_in[:D], b_in[D:2 * D], b_in[2 * D:]
    d["wqT"] = to8(wq.T)
    d["wkT"] = to8(wk.T)
    d["wvT"] = np.ascontiguousarray(
        ((wv * ln["ln2_g"][None, :]).T).astype(ml_dtypes.bfloat16))
    d["bq"] = colvec(bq, 4)
    d["bk"] = colvec(bk, 4)
    d["bv_row"] = np.ascontiguousarray((bv + ln["ln2_b"] @ wv.T).reshape(1, D))
    w_out = np.asarray(inp["w_out"], f32)
    d["woutTh"] = np.ascontiguousarray(
        w_out.T.reshape(H, HD, D).transpose(1, 0, 2)
        .astype(ml_dtypes.bfloat16))                           # [64, 8, 512]
    d["bout"] = colvec(np.asarray(inp["b_out"], f32), 4)

    pos = np.arange(L, dtype=f32)
    num = np.arange(0, D, 2, dtype=f32) / D
    scale = (1.0 / ROPE_BASE ** num).astype(ml_dtypes.bfloat16).astype(f32)
    theta = pos[:, None] * scale[None, :]                      # [512, 256]
    cosT, sinT = np.cos(theta).T, np.sin(theta).T              # [256, 512]
    cos2 = np.concatenate([cosT] * BL, axis=1)                 # [256, 1024]
    sin2 = np.concatenate([sinT] * BL, axis=1)
    g1h = ln["ln2_g"][:D // 2][:, None]
    g2h = ln["ln2_g"][D // 2:][:, None]
    ropetab = np.stack(
        [g1h * cos2, g1h * sin2, g2h * cos2, g2h * sin2])      # [4, 256, T]
    # device layout [128, 4, 2, T] (partition-major, contiguous DMA); bf16
    # so the rope tensor_tensor chain runs in the DVE 2x packed mode
    d["ropetab"] = np.ascontiguousarray(
        ropetab.reshape(4, 2, 128, T).transpose(2, 0, 1, 3)
        .astype(ml_dtypes.bfloat16))

    has_qkfix = not np.allclose(ln["ln2_b"], 0.0)
    if has_qkfix:
        bb = np.broadcast_to(ln["ln2_b"], (L, D)).astype(f32)
        half = D // 2
        rb = np.concatenate([bb[:, :half] * cosT.T - bb[:, half:] * sinT.T,
                             bb[:, half:] * cosT.T + bb[:, :half] * sinT.T], axis=1)
        qfix = np.concatenate([(rb @ wq.T).T] * BL, axis=1)
        kfix = np.concatenate([(rb @ wk.T).T] * BL, axis=1)
        d["qkfix"] = np.ascontiguousarray(np.stack([qfix, kfix]))  # [2, 512, 1024]

    w_glu = np.asarray(inp["w_glu"], f32)
    # a-half scaled x16 so conv_in (fp8) sits higher above the subnormal
    # cutoff; BN downstream absorbs the uniform scale exactly.
    CINS = 16.0
    wglu_s = w_glu * ln["ln3_g"][None, :]
    wglu_s[:D] *= CINS
    d["wgluT"] = np.ascontiguousarray(wglu_s.T.astype(ml_dtypes.bfloat16))
    bglu_s = ln["ln3_b"] @ w_glu.T
    bglu_s[:D] *= CINS
    d["bglu"] = colvec(bglu_s, 8)
    w_dw = np.asarray(inp["w_dw"], f32)[:, 0, :]               # [512, 31]
    # per-channel power-of-2 scale into fp8 range; BN normalization
    # absorbs any per-channel scale on the conv output exactly.
    cmax = np.abs(w_dw).max(axis=1)
    cscl = np.exp2(np.floor(np.log2(128.0 / np.maximum(cmax, 1e-30))))
    w_dws = w_dw * cscl[:, None]
    diag = np.zeros((NC, 128, KTAP2, 128), f32)
    idx = np.arange(128)
    for c in range(NC):
        diag[c, idx, :KTAP, idx] = w_dws[128 * c + idx, :]
    d["diag"] = np.ascontiguousarray(np.clip(diag, -224, 224).astype(NP8))
    d["bng"] = colvec(np.asarray(inp["bn_g"], f32), 4)
    d["bnb"] = colvec(np.asarray(inp["bn_b"], f32), 4)
    d["wpwT"] = np.ascontiguousarray(
        np.asarray(inp["w_pw"], f32).T.astype(ml_dtypes.bfloat16))
    d["eye"] = np.eye(128, dtype=f32)
    d["cconst"] = np.full((128, 1), 1.0 / D, f32)
    d["rowones"] = np.ones((1, 128), f32)
    d["rowones_bf"] = np.ones((1, 128), ml_dtypes.bfloat16)
    d["padzero"] = np.zeros((128, NC, PAD), NP8)

    ln5_nontrivial = not (np.allclose(ln["ln5_g"], 1.0)
                          and np.allclose(ln["ln5_b"], 0.0))
    if ln5_nontrivial:
        d["g5"] = colvec(ln["ln5_g"], 4)
        d["b5"] = colvec(ln["ln5_b"], 4)

    xs = []
    for c in range(N_CORES):
        xc = x[BL * c: BL * (c + 1)]                           # [2, 512, 512]
        xs.append(np.ascontiguousarray(xc.transpose(2, 0, 1).reshape(D, T)))
    return d, xs, (has_qkfix, ln5_nontrivial, has_bffa, has_bffb)


# ---------------------------------------------------------------- device build

def _build(flags):
    has_qkfix, has_ln5gb, has_bffa, has_bffb = flags
    nc = bacc.Bacc("TRN2", target_bir_lowering=False, debug=False,
                   enable_asserts=True, num_devices=N_CORES)
    AOT = mybir.AluOpType
    AF = mybir.ActivationFunctionType
    DR = mybir.MatmulPerfMode.DoubleRow

    def din(name, shape, dt=FP32):
        return nc.dram_tensor(name, list(shape), dt, kind="ExternalInput")

    x_in = din("x_fm", [D, T])
    wff1a_d = din("wff1a", [D, FF], FP8); bff1a_d = din("bff1a", [128, 16])
    wff1b_d = din("wff1b", [FF, D], FP8); bff1b_d = din("bff1b", [128, 4])
    wff2a_d = din("wff2a", [D, FF], FP8); bff2a_d = din("bff2a", [128, 16])
    wff2b_d = din("wff2b", [FF, D], FP8); bff2b_d = din("bff2b", [128, 4])
    wqT_d = din("wqT", [D, D], FP8); wkT_d = din("wkT", [D, D], FP8)
    wvT_d = din("wvT", [D, D], BF16)
    bq_d = din("bq", [128, 4]); bk_d = din("bk", [128, 4]); bv_d = din("bv_row", [1, D])
    woutTh_d = din("woutTh", [HD, H, D], BF16); bout_d = din("bout", [128, 4])
    ropetab_d = din("ropetab", [128, 4, 2, T], BF16)
    wgluT_d = din("wgluT", [D, 2 * D], BF16); bglu_d = din("bglu", [128, 8])
    diag_d = din("diag", [NC, 128, KTAP2, 128], FP8)
    bng_d = din("bng", [128, 4]); bnb_d = din("bnb", [128, 4])
    wpwT_d = din("wpwT", [D, D], BF16)
    eye_d = din("eye", [128, 128])
    cconst_d = din("cconst", [128, 1])
    rowones_d = din("rowones", [1, 128])
    rowones_bf_d = din("rowones_bf", [1, 128], BF16)
    padzero_d = din("padzero", [128, NC, PAD], FP8)
    qkfix_d = din("qkfix", [2, D, T]) if has_qkfix else None
    g5_d = din("g5", [128, 4]) if has_ln5gb else None
    b5_d = din("b5", [128, 4]) if has_ln5gb else None
    out_d = nc.dram_tensor("out", [BL, L, D], FP32, kind="ExternalOutput")
    out_flat = out_d.ap().rearrange("b l d -> (b l) d")

    def chunked(ap_dram):
        return ap_dram.ap().rearrange("(c p) f -> p c f", p=128)

    with tile.TileContext(nc) as tc:
        ctx = contextlib.ExitStack()
        with ctx:
            resid = ctx.enter_context(tc.tile_pool(name="resid", bufs=1))
            zpool = ctx.enter_context(tc.tile_pool(name="zpool", bufs=1))
            scr = ctx.enter_context(tc.tile_pool(name="scr", bufs=1))
            sqp = ctx.enter_context(tc.tile_pool(name="sqp", bufs=2))
            stat = ctx.enter_context(tc.tile_pool(name="stat", bufs=1))
            bias1 = ctx.enter_context(tc.tile_pool(name="bias1", bufs=1))
            dpool = ctx.enter_context(tc.tile_pool(name="dpool", bufs=4, space="DRAM"))

            # ---------------- persistent tiles ----------------
            # small constants FIRST in the DMA queue: the PE warmup burst
            # depends only on eye_r, so it must not queue behind the 2MB x
            # transfer (that stalls the whole PE FIFO and cools the HAM).
            oneD_r = bias1.tile([128, 1], FP32R, tag="oneD_r")
            nc.sync.dma_start(oneD_r[:], cconst_d.ap().bitcast(FP32R))
            ones_row_r = bias1.tile([1, 128], FP32R, tag="ones_row")
            nc.sync.dma_start(ones_row_r[:], rowones_d.ap().bitcast(FP32R))
            ones_row64 = bias1.tile([HD + 1, 128], BF16, tag="ones_row64")
            nc.sync.dma_start(ones_row64[HD:HD + 1, :], rowones_bf_d.ap())
            eye_r = bias1.tile([128, 128], FP32R, tag="eye_r")
            nc.sync.dma_start(eye_r[:], eye_d.ap().bitcast(FP32R))
            eye_sb = bias1.tile([128, 128], FP32, tag="eye_sb")
            nc.sync.dma_start(eye_sb[:], eye_d.ap())
            x = resid.tile([128, NC, T], FP32R, tag="x")
            for _c in range(NC):
                nc.sync.dma_start(x[:, _c, :], chunked(x_in).bitcast(FP32R)[:, _c, :])
            eps_sb = bias1.tile([128, 1], FP32, tag="eps")
            nc.vector.memset(eps_sb[:], LN_EPS)
            rows_sb = bias1.tile([128, 1024], FP32, tag="rows")
            nc.vector.memset(rows_sb[:], 0.0)

            # ---------------- FFN1 weights (prefetch from t=0) ----------------
            wp1_cm = tc.tile_pool(name="wp1", bufs=1)
            wp1 = wp1_cm.__enter__()

            def load_ffn_w(wpool, wa_d, ba_d, wb_d, bb_d, tag):
                wa = wpool.tile([128, NC, FF], FP8, tag="wa" + tag)
                for _m in range(0, 16, 8):
                    nc.sync.dma_start(
                        wa[:, :, 128 * _m:128 * (_m + 8)],
                        chunked(wa_d)[:, :, 128 * _m:128 * (_m + 8)])
                wb = wpool.tile([128, 16, D], FP8, tag="wb" + tag)
                _wbap = wb_d.ap().rearrange("(c p) f -> p c f", p=128)
                for _k in range(0, 16, 8):
                    nc.sync.dma_start(wb[:, _k:_k + 8, :], _wbap[:, _k:_k + 8, :])
                ba = bias1.tile([128, 16], FP32, tag="ba" + tag)
                nc.sync.dma_start(ba[:], ba_d.ap())
                bb = bias1.tile([128, 4], FP32, tag="bb" + tag)
                nc.sync.dma_start(bb[:], bb_d.ap())
                return wa, wb, ba, bb

            w1 = load_ffn_w(wp1, wff1a_d, bff1a_d, wff1b_d, bff1b_d, "1")

            # ---------------- PE warmup (HAM K=8/8) ----------------
            # ~16 x 414ns fp32r matmuls = ~6.6us of PE busy, enough for one
            # HAM SHORT window; ends about when the x DMA lands so LN1
            # stats are not blocked behind it.
            with tc.tile_pool(name="pswm", bufs=1, space="PSUM") as pswm:
                wm_ps = pswm.tile([128, 128], FP32, tag="wm")
                for _i in range(16):
                    nc.tensor.matmul(wm_ps[:], eye_r[:], eye_r[:],
                                     start=True, stop=True)

            # ---------------- collective warm-up ----------------
            warm_sb = bias1.tile([128, 8], FP32, tag="warm")
            nc.vector.memset(warm_sb[:], 0.0)
            warm_in = dpool.tile([128, 8], FP32)
            warm_out = dpool.tile([128, 8], FP32)
            nc.gpsimd.dma_start(warm_in[:], warm_sb[:])
            nc.gpsimd.collective_compute(
                "AllReduce", AOT.add, replica_groups=[list(range(N_CORES))],
                ins=[warm_in.opt()], outs=[warm_out.opt()])
            nc.gpsimd.dma_start(warm_sb[:], warm_out[:])

            def act_reciprocal(out, in_):
                """ACT-engine reciprocal (bass guards it for accuracy; ~1e-5
                rel err measured on HW -- plenty for softmax denominators)."""
                bias = nc.scalar.bass.const_aps.scalar_like(0.0, in_)
                inputs = [nc.scalar.lower_ap(in_), nc.scalar.lower_ap(bias),
                          mybir.ImmediateValue(dtype=mybir.dt.float32, value=1.0),
                          mybir.ImmediateValue(dtype=mybir.dt.float32, value=0.0)]
                return nc.scalar.add_instruction(mybir.InstActivation(
                    name=nc.scalar.bass.get_next_instruction_name(),
                    func=AF.Reciprocal, ins=inputs,
                    outs=[nc.scalar.lower_ap(out)]))

            # ---------------- layer norm helper ----------------
            def layer_norm(tag, apply=True, zdt=BF16):
                """Standardize x over features.  Processed per token-half so
                the second half's stats overlap the first half's apply:
                row stats via matmul -> PE transpose -> 128-lane math ->
                stride-0 matmul broadcast -> 2 DVE ops per chunk-half."""
                sttag = "st5" if not apply else "st"
                rtag = "rt5" if not apply else "rt"
                st_sb = stat.tile([128, 8, 2], FP32, tag=sttag)
                r_tok = stat.tile([128, 8], FP32R, tag=rtag)
                mr_tok = None
                z = None
                tmp = None
                if apply:
                    mr_tok = stat.tile([128, 8], FP32R, tag="mrt")
                    z = zpool.tile([128, NC, T], zdt, tag="z")
                    tmp = scr.tile([128, T], FP32, tag="lntmp")
                with tc.tile_pool(name="psln" + tag, bufs=1, space="PSUM") as psln, \
                     tc.tile_pool(name="pstp" + tag, bufs=2, space="PSUM") as pstp, \
                     tc.tile_pool(name="psbc" + tag, bufs=1, space="PSUM") as psbc:
                    for nh in range(2):
                        sl = slice(512 * nh, 512 * (nh + 1))
                        mean_ps = psln.tile([1, 512], FP32, tag="mean")
                        msq_ps = psln.tile([1, 512], FP32, tag="msq")
                        for c in range(NC):
                            sq = sqp.tile([128, 512], FP32R, tag="sq")
                            nc.scalar.activation(
                                out=sq[:], in_=x[:, c, sl].bitcast(FP32),
                                func=AF.Square)
                            nc.tensor.matmul(mean_ps[:], oneD_r[:], x[:, c, sl],
                                             start=(c == 0), stop=(c == NC - 1))
                            nc.tensor.matmul(msq_ps[:], oneD_r[:], sq[:],
                                             start=(c == 0), stop=(c == NC - 1))
                        nc.vector.tensor_copy(rows_sb[0:1, sl], mean_ps[:])
                        nc.vector.tensor_copy(rows_sb[32:33, sl], msq_ps[:])
                        hb = slice(4 * nh, 4 * (nh + 1))
                        for tb in range(4 * nh, 4 * (nh + 1)):
                            st_ps = pstp.tile([128, 128], FP32, tag="stp")
                            nc.tensor.transpose(st_ps[:],
                                                rows_sb[:, 128 * tb:128 * (tb + 1)],
                                                eye_sb[:])
                            nc.vector.tensor_copy(
                                st_sb[:, tb, :],
                                st_ps[:, 0:64].rearrange(
                                    "p (a b) -> p a b", a=2)[:, :, 0])
                        m2 = stat.tile([128, 4], FP32, tag="m2t")
                        nc.scalar.activation(out=m2[:], in_=st_sb[:, hb, 0],
                                             func=AF.Square)
                        sig = stat.tile([128, 4], FP32, tag="sigt")
                        nc.vector.tensor_tensor(out=sig[:], in0=st_sb[:, hb, 1],
                                                in1=m2[:], op=AOT.subtract)
                        nc.scalar.activation(out=sig[:], in_=sig[:], func=AF.Sqrt,
                                             bias=eps_sb[:], scale=1.0)
                        with nc.allow_low_precision(reason="fp32r is rounded fp32"):
                            nc.vector.reciprocal(out=r_tok[:, hb], in_=sig[:])
                        if not apply:
                            continue
                        nc.vector.tensor_tensor(out=mr_tok[:, hb],
                                                in0=st_sb[:, hb, 0],
                                                in1=r_tok[:, hb].bitcast(FP32),
                                                op=AOT.mult)
                        bc_r = psbc.tile([128, 512], FP32, tag="bcr")
                        bc_mr = psbc.tile([128, 512], FP32, tag="bcmr")
                        for tb in range(4):
                            nc.tensor.matmul(
                                bc_r[:, 128 * tb:128 * (tb + 1)],
                                r_tok[:, 4 * nh + tb:4 * nh + tb + 1]
                                .to_broadcast((128, 128)),
                                eye_r[:], start=True, stop=True)
                            nc.tensor.matmul(
                                bc_mr[:, 128 * tb:128 * (tb + 1)],
                                mr_tok[:, 4 * nh + tb:4 * nh + tb + 1]
                                .to_broadcast((128, 128)),
                                eye_r[:], start=True, stop=True)
                        for c in range(NC):
                            nc.vector.tensor_tensor(
                                out=tmp[:, sl], in0=x[:, c, sl].bitcast(FP32),
                                in1=bc_r[:], op=AOT.mult)
                            nc.vector.tensor_tensor(
                                out=z[:, c, sl], in0=tmp[:, sl],
                                in1=bc_mr[:], op=AOT.subtract)
                if not apply:
                    return st_sb, r_tok
                return z

            def ffn(z, w, tag):
                wa, wb, ba, bb = w
                with tc.tile_pool(name="hp" + tag, bufs=1) as hpool, \
                     tc.tile_pool(name="psf" + tag, bufs=2, space="PSUM") as psum:
                    for nh in range(2):
                        hid = hpool.tile([128, 16, 512], FP8, tag="hid")
                        for mt in range(16):
                            p = psum.tile([128, 512], FP32, tag="mm")
                            for kc in range(0, NC, 2):
                                nc.tensor.matmul(
                                    p[:], wa[:, kc:kc + 2, 128 * mt:128 * (mt + 1)],
                                    z[:, kc:kc + 2, 512 * nh:512 * (nh + 1)],
                                    start=(kc == 0), stop=(kc == NC - 2),
                                    perf_mode=DR)
                            nc.scalar.activation(
                                out=hid[:, mt, :], in_=p[:],
                                func=AF.Relu, bias=ba[:, mt:mt + 1], scale=ISCL)
                        for mc in range(NC):
                            p = psum.tile([128, 512], FP32, tag="mm")
                            for kt in range(0, 16, 2):
                                nc.tensor.matmul(
                                    p[:], wb[:, kt:kt + 2, 128 * mc:128 * (mc + 1)],
                                    hid[:, kt:kt + 2, :],
                                    start=(kt == 0), stop=(kt == 14),
                                    perf_mode=DR)
                            if has_bffb:
                                tmp_b = hpool.tile([128, 512], FP32, tag="tb")
                                nc.vector.tensor_scalar(
                                    out=tmp_b[:], in0=p[:], scalar1=ISCL,
                                    scalar2=bb[:, mc:mc + 1],
                                    op0=AOT.mult, op1=AOT.add)
                                nc.vector.tensor_tensor(
                                    out=x[:, mc, 512 * nh:512 * (nh + 1)],
                                    in0=tmp_b[:],
                                    in1=x[:, mc, 512 * nh:512 * (nh + 1)].bitcast(FP32),
                                    op=AOT.add)
                            else:
                                nc.vector.scalar_tensor_tensor(
                                    out=x[:, mc, 512 * nh:512 * (nh + 1)],
                                    in0=p[:], scalar=ISCL,
                                    in1=x[:, mc, 512 * nh:512 * (nh + 1)].bitcast(FP32),
                                    op0=AOT.mult, op1=AOT.add)

            # ================= FFN1 =================
            with nc.named_scope("ffn1"):
                z1 = layer_norm("1", zdt=FP8)
                ffn(z1, w1, "1")
            wp1_cm.__exit__(None, None, None)

            # ---- conv_in + GLU weights: reserve + prefetch before attention ----
            # conv_in[:, 0] = GLU output (E copy); conv_in[:, 1] = same data
            # shifted left one column (O copy) so DoubleRow tap pairs
            # (2p, 2p+1) read [128, 2, 512] with a standard strided AP.
            convpre = ctx.enter_context(tc.tile_pool(name="convpre", bufs=1))
            conv_in = convpre.tile([128, 2, NC, CONVW], FP8, tag="cin")
            pz = padzero_d.ap()
            nc.sync.dma_start(conv_in[:, 0, :, 0:PAD], pz)
            nc.sync.dma_start(conv_in[:, 0, :, PAD + L:2 * PAD + L], pz)
            nc.sync.dma_start(conv_in[:, 0, :, 2 * PAD + 2 * L:CONVW], pz)
            nc.sync.dma_start(conv_in[:, 1, :, CONVW - 1:CONVW], pz[:, :, 0:1])
            wg_sb = convpre.tile([128, NC, 2 * D], BF16, tag="wg")
            nc.sync.dma_start(wg_sb[:], chunked(wgluT_d))
            bg_sb = bias1.tile([128, 8], FP32, tag="bg")
            nc.sync.dma_start(bg_sb[:], bglu_d.ap())

            # ================= attention =================
            with nc.named_scope("attn"):
                z2 = layer_norm("2")
                with tc.tile_pool(name="apool", bufs=1) as apool:
                    # bf16 q / zero-padded k (K=128 keeps the PE array fully
                    # active for scores; the zero half kills the other head)
                    q_sb = apool.tile([128, NC, T], BF16, tag="q")
                    k_ev = apool.tile([128, NC, T], BF16, tag="kev")
                    k_od = apool.tile([128, NC, T], BF16, tag="kod")
                    nc.vector.tensor_scalar(
                        out=k_ev[64:128, :, :], in0=x[64:128, :, :].bitcast(FP32),
                        scalar1=0.0, scalar2=0.0, op0=AOT.mult, op1=AOT.add)
                    nc.vector.tensor_scalar(
                        out=k_od[0:64, :, :], in0=x[0:64, :, :].bitcast(FP32),
                        scalar1=0.0, scalar2=0.0, op0=AOT.mult, op1=AOT.add)
                    v_aug = apool.tile([128, 8, H, HD + 1], BF16, tag="vaug")
                    nc.vector.tensor_scalar(
                        out=v_aug[:, :, :, HD:HD + 1],
                        in0=eye_sb[:, 0:64].rearrange(
                            "p (a b) -> p a b", a=8)[:, :, :, None],
                        scalar1=0.0, scalar2=1.0, op0=AOT.mult, op1=AOT.add)
                    with tc.tile_pool(name="wvp", bufs=1) as wvp, \
                         tc.tile_pool(name="psa", bufs=3, space="PSUM") as psum:
                        # ---- v projection (token-major, ones-augmented) ----
                        wv_sb = wvp.tile([128, NC, D], BF16, tag="wv")
                        nc.sync.dma_start(wv_sb[:], chunked(wvT_d))
                        bvr = bias1.tile([1, D], FP32R, tag="bvr")
                        nc.sync.dma_start(bvr[:], bv_d.ap().bitcast(FP32R))
                        for tt in range(8):
                            p = psum.tile([128, 512], FP32, tag="mm")
                            for kc in range(NC):
                                nc.tensor.matmul(p[:], z2[:, kc, 128 * tt:128 * (tt + 1)],
                                                 wv_sb[:, kc, :],
                                                 start=(kc == 0), stop=False)
                            nc.tensor.matmul(p[:], ones_row_r[:], bvr[:],
                                             start=False, stop=True)
                            nc.scalar.copy(out=v_aug[:, tt, :, 0:HD],
                                           in_=p[:].rearrange("p (h f) -> p h f", h=H))
                        # PE keep-warm while the rope DVE chain runs
                        with tc.tile_pool(name="pswa", bufs=1, space="PSUM") as pswa:
                            junk_a = pswa.tile([128, 512], FP32, tag="jpa")
                            for _i in range(24):
                                nc.tensor.matmul(junk_a[:], eye_r[:],
                                                 x[:, 0, 0:512],
                                                 start=True, stop=True)
                        # ---- rope + q/k ----
                        if has_qkfix:
                            qkf = apool.tile([128, 2, NC, T], FP32, tag="qkf")
                            nc.sync.dma_start(
                                qkf[:],
                                qkfix_d.ap().rearrange("k (c p) t -> p k c t", p=128))
                        with tc.tile_pool(name="hrp", bufs=1) as hrp:
                            hr = hrp.tile([128, NC, T], FP8, tag="hr")
                            with tc.tile_pool(name="tabp", bufs=1) as tabp:
                                tab = tabp.tile([128, 4, 2, T], BF16, tag="ropetab")
                                nc.sync.dma_start(tab[:], ropetab_d.ap())
                                rtmp = tabp.tile([128, T], BF16, tag="rtmp")
                                rtmp2 = tabp.tile([128, T], BF16, tag="rtmp2")
                                for c in range(2):
                                    nc.vector.tensor_tensor(
                                        out=rtmp[:], in0=z2[:, c, :],
                                        in1=tab[:, 0, c, :], op=AOT.mult)
                                    nc.vector.tensor_tensor(
                                        out=rtmp2[:], in0=z2[:, c + 2, :],
                                        in1=tab[:, 3, c, :], op=AOT.mult)
                                    nc.vector.tensor_tensor(
                                        out=hr[:, c, :], in0=rtmp[:], in1=rtmp2[:],
                                        op=AOT.subtract)
                                    nc.vector.tensor_tensor(
                                        out=rtmp[:], in0=z2[:, c + 2, :],
                                        in1=tab[:, 2, c, :], op=AOT.mult)
                                    nc.vector.tensor_tensor(
                                        out=rtmp2[:], in0=z2[:, c, :],
                                        in1=tab[:, 1, c, :], op=AOT.mult)
                                    nc.vector.tensor_tensor(
                                        out=hr[:, c + 2, :], in0=rtmp[:], in1=rtmp2[:],
                                        op=AOT.add)
                            with tc.tile_pool(name="wqkp", bufs=1) as wqkp:
                                wq_sb = wqkp.tile([128, NC, D], FP8, tag="wq")
                                wk_sb = wqkp.tile([128, NC, D], FP8, tag="wk")
                                nc.sync.dma_start(wq_sb[:], chunked(wqT_d))
                                nc.sync.dma_start(wk_sb[:], chunked(wkT_d))
                                bqs = bias1.tile([128, 4], FP32, tag="bqs")
                                bks = bias1.tile([128, 4], FP32, tag="bks")
                                nc.sync.dma_start(bqs[:], bq_d.ap())
                                nc.sync.dma_start(bks[:], bk_d.ap())
                                qk_tmp = None
                                if has_qkfix:
                                    qk_tmp = wqkp.tile([128, 512], FP32, tag="qkt")
                                for mt in range(NC):
                                    for nh in range(2):
                                        p = psum.tile([128, 512], FP32, tag="mm")
                                        for kc in range(0, NC, 2):
                                            nc.tensor.matmul(
                                                p[:], wq_sb[:, kc:kc + 2,
                                                            128 * mt:128 * (mt + 1)],
                                                hr[:, kc:kc + 2,
                                                   512 * nh:512 * (nh + 1)],
                                                start=(kc == 0), stop=(kc == NC - 2),
                                                perf_mode=DR)
                                        if has_qkfix:
                                            nc.vector.tensor_scalar(
                                                out=qk_tmp[:], in0=p[:],
                                                scalar1=ISCL,
                                                scalar2=bqs[:, mt:mt + 1],
                                                op0=AOT.mult, op1=AOT.add)
                                            nc.vector.tensor_tensor(
                                                out=q_sb[:, mt, 512 * nh:512 * (nh + 1)],
                                                in0=qk_tmp[:],
                                                in1=qkf[:, 0, mt,
                                                        512 * nh:512 * (nh + 1)],
                                                op=AOT.add)
                                        else:
                                            nc.scalar.activation(
                                                out=q_sb[:, mt, 512 * nh:512 * (nh + 1)],
                                                in_=p[:], func=AF.Identity,
                                                bias=bqs[:, mt:mt + 1], scale=ISCL)
                                for mt in range(NC):
                                    for nh in range(2):
                                        p = psum.tile([128, 512], FP32, tag="mm")
                                        for kc in range(0, NC, 2):
                                            nc.tensor.matmul(
                                                p[:], wk_sb[:, kc:kc + 2,
                                                            128 * mt:128 * (mt + 1)],
                                                hr[:, kc:kc + 2,
                                                   512 * nh:512 * (nh + 1)],
                                                start=(kc == 0), stop=(kc == NC - 2),
                                                perf_mode=DR)
                                        if has_qkfix:
                                            nc.vector.tensor_scalar(
                                                out=qk_tmp[:], in0=p[:],
                                                scalar1=ISCL,
                                                scalar2=bks[:, mt:mt + 1],
                                                op0=AOT.mult, op1=AOT.add)
                                            nc.vector.tensor_tensor(
                                                out=k_ev[0:64, mt, 512 * nh:512 * (nh + 1)],
                                                in0=qk_tmp[0:64, :],
                                                in1=qkf[0:64, 1, mt,
                                                        512 * nh:512 * (nh + 1)],
                                                op=AOT.add)
                                            nc.vector.tensor_tensor(
                                                out=k_od[64:128, mt, 512 * nh:512 * (nh + 1)],
                                                in0=qk_tmp[64:128, :],
                                                in1=qkf[64:128, 1, mt,
                                                        512 * nh:512 * (nh + 1)],
                                                op=AOT.add)
                                        else:
                                            nc.scalar.activation(
                                                out=k_ev[0:64, mt, 512 * nh:512 * (nh + 1)],
                                                in_=p[0:64, :], func=AF.Identity,
                                                bias=bks[0:64, mt:mt + 1], scale=ISCL)
                                            nc.scalar.activation(
                                                out=k_od[64:128, mt, 512 * nh:512 * (nh + 1)],
                                                in_=p[64:128, :], func=AF.Identity,
                                                bias=bks[64:128, mt:mt + 1], scale=ISCL)
                    # ---- scores / AV: software-pipelined, one Exp per head ----
                    with tc.tile_pool(name="wop", bufs=1) as wop, \
                         tc.tile_pool(name="osbp", bufs=2) as osbp:
                        wo_sb = wop.tile([HD, H, D], BF16, tag="wo")
                        nc.sync.dma_start(wo_sb[:], woutTh_d.ap())
                        bo_sb = bias1.tile([128, 4], FP32, tag="bo")
                        nc.sync.dma_start(bo_sb[:], bout_d.ap())
                        o_raws = []
                        for b in range(BL):
                            o_raw = osbp.tile([HD + 1, H, 512], BF16, tag="osb")
                            o_raws.append(o_raw)
                        iters = [(h, b) for h in range(H) for b in range(BL)]
                        SKEW = 1
                        ets = {}
                        with tc.tile_pool(name="pss", bufs=3, space="PSUM") as pss, \
                             tc.tile_pool(name="psav", bufs=2, space="PSUM") as psav, \
                             tc.tile_pool(name="ep", bufs=3) as epool:
                            for i in range(len(iters) + SKEW):
                                if i < len(iters):
                                    h, b = iters[i]
                                    kp = k_ev if h % 2 == 0 else k_od
                                    ch = h // 2
                                    e_t = epool.tile([128, 4, 512], BF16, tag="e")
                                    for half in range(2):
                                        s_ps = pss.tile([128, 2, 512], FP32, tag="sps")
                                        for kk in range(2):
                                            kt = 2 * half + kk
                                            nc.tensor.matmul(
                                                s_ps[:, kk, :],
                                                kp[:, ch, 512 * b + 128 * kt:
                                                   512 * b + 128 * (kt + 1)],
                                                q_sb[:, ch, 512 * b:512 * (b + 1)],
                                                start=True, stop=True)
                                        nc.scalar.activation(
                                            out=e_t[:, 2 * half:2 * half + 2, :]
                                            .rearrange("p a b -> p (a b)"),
                                            in_=s_ps[:].rearrange("p a b -> p (a b)"),
                                            func=AF.Exp, scale=1.0 / 8.0)
                                    ets[i] = e_t
                                if i >= SKEW:
                                    h, b = iters[i - SKEW]
                                    e_t = ets.pop(i - SKEW)
                                    o_ps = psav.tile([HD + 1, 512], FP32, tag="avo")
                                    for kt in range(4):
                                        nc.tensor.matmul(o_ps[:],
                                                         v_aug[:, 4 * b + kt, h, :],
                                                         e_t[:, kt, :],
                                                         start=(kt == 0), stop=(kt == 3))
                                    with nc.allow_low_precision(reason="bf16 attn"):
                                        nc.vector.tensor_copy(o_raws[b][:, h, :], o_ps[:])
                        # ---- normalization + out-proj ----
                        with tc.tile_pool(name="psbch", bufs=2, space="PSUM") as psbch, \
                             tc.tile_pool(name="pso", bufs=3, space="PSUM") as pso:
                            for b in range(BL):
                                act_reciprocal(
                                    o_raws[b][HD:HD + 1, :, :].rearrange(
                                        "p h t -> p (h t)"),
                                    o_raws[b][HD:HD + 1, :, :].rearrange(
                                        "p h t -> p (h t)"))
                                for h in range(H):
                                    bch = psbch.tile([HD, 512], FP32, tag="bch")
                                    nc.tensor.matmul(bch[:],
                                                     ones_row64[HD:HD + 1, 0:HD],
                                                     o_raws[b][HD:HD + 1, h, :],
                                                     start=True, stop=True)
                                    with nc.allow_low_precision(reason="bf16 attn"):
                                        nc.vector.tensor_tensor(
                                            out=o_raws[b][0:HD, h, :],
                                            in0=o_raws[b][0:HD, h, :],
                                            in1=bch[:], op=AOT.mult)
                                for mc in range(NC):
                                    p = pso.tile([128, 512], FP32, tag="mm")
                                    for h in range(H):
                                        nc.tensor.matmul(
                                            p[:], wo_sb[:, h, 128 * mc:128 * (mc + 1)],
                                            o_raws[b][0:HD, h, :],
                                            start=(h == 0), stop=(h == H - 1))
                                    nc.vector.scalar_tensor_tensor(
                                        out=x[:, mc, 512 * b:512 * (b + 1)],
                                        in0=p[:], scalar=bo_sb[:, mc:mc + 1],
                                        in1=x[:, mc, 512 * b:512 * (b + 1)].bitcast(FP32),
                                        op0=AOT.add, op1=AOT.add)

            # ---- FFN2 weights: reserve + prefetch during conv ----
            wp2 = ctx.enter_context(tc.tile_pool(name="wp2", bufs=1))
            w2 = load_ffn_w(wp2, wff2a_d, bff2a_d, wff2b_d, bff2b_d, "2")

            # ================= conv module =================
            with nc.named_scope("conv"):
                z3 = layer_norm("3")
                bng_sb = bias1.tile([128, 4], FP32, tag="bngw")
                bnb_sb = bias1.tile([128, 4], FP32, tag="bnbw")
                nc.sync.dma_start(bng_sb[:], bng_d.ap())
                nc.sync.dma_start(bnb_sb[:], bnb_d.ap())
                ntot = float(N_CORES * T)
                with tc.tile_pool(name="cpool", bufs=1) as cpool:
                    conv_raw = cpool.tile([128, NC, T], BF16, tag="craw")
                    st6 = stat.tile([128, NC, 2, 6], FP32, tag="st6")
                    stats_loc = stat.tile([128, NC, 2], FP32, tag="bnloc")
                    gstats = stat.tile([128, NC, 2], FP32, tag="bngl")
                    # per-chunk: GLU -> E/O copy -> dw conv -> local stats;
                    # one batched all-reduce after the last chunk
                    with tc.tile_pool(name="sigp", bufs=2) as sigp, \
                         tc.tile_pool(name="diagp", bufs=2) as diagp, \
                         tc.tile_pool(name="psg", bufs=2, space="PSUM") as psg, \
                         tc.tile_pool(name="psc", bufs=2, space="PSUM") as psc:
                        for c in range(NC):
                            for nh in range(2):
                                p_a = psg.tile([128, 512], FP32, tag="pa")
                                p_s = psg.tile([128, 512], FP32, tag="psg")
                                for kc in range(NC):
                                    nc.tensor.matmul(
                                        p_s[:], wg_sb[:, kc,
                                                      128 * (c + 4):128 * (c + 5)],
                                        z3[:, kc, 512 * nh:512 * (nh + 1)],
                                        start=(kc == 0), stop=(kc == NC - 1))
                                for kc in range(NC):
                                    nc.tensor.matmul(
                                        p_a[:], wg_sb[:, kc, 128 * c:128 * (c + 1)],
                                        z3[:, kc, 512 * nh:512 * (nh + 1)],
                                        start=(kc == 0), stop=(kc == NC - 1))
                                sig = sigp.tile([128, 512], FP32, tag="sig")
                                nc.scalar.activation(out=sig[:], in_=p_s[:],
                                                     func=AF.Sigmoid,
                                                     bias=bg_sb[:, c + 4:c + 5],
                                                     scale=1.0)
                                nc.vector.scalar_tensor_tensor(
                                    out=conv_in[:, 0, c, OFF_B[nh]:OFF_B[nh] + L],
                                    in0=p_a[:], scalar=bg_sb[:, c:c + 1],
                                    in1=sig[:], op0=AOT.add, op1=AOT.mult)
                            # O copy = E shifted one column left (pads carry
                            # over); on the gpsimd queue so its wait on the
                            # GLU STT doesn't head-of-line-block the sync
                            # queue's weight prefetches
                            nc.gpsimd.dma_start(conv_in[:, 1, c, 0:CONVW - 1],
                                                conv_in[:, 0, c, 1:CONVW])
                            # dw conv: 16 DoubleRow tap-pairs per (chunk, batch)
                            diag = diagp.tile([128, KTAP2, 128], FP8, tag="diag")
                            nc.sync.dma_start(diag[:], diag_d.ap()[c])
                            cps = []
                            for _b in range(BL):
                                cpb = psc.tile([128, 512], FP32, tag="cps")
                                cps.append(cpb)
                            for pp in range(KTAP2 // 2):
                                for b in range(BL):
                                    s0 = OFF_B[b] - PAD + 2 * pp
                                    nc.tensor.matmul(
                                        cps[b][:], diag[:, 2 * pp:2 * pp + 2, :],
                                        conv_in[:, :, c, s0:s0 + L],
                                        start=(pp == 0), stop=(pp == KTAP2 // 2 - 1),
                                        perf_mode=DR)
                            for b in range(BL):
                                nc.vector.bn_stats(out=st6[:, c, b, :], in_=cps[b][:])
                                nc.scalar.copy(out=conv_raw[:, c, L * b:L * (b + 1)],
                                               in_=cps[b][:])
                            # local stats (scaled to sums)
                            mv = stat.tile([128, 2], FP32, tag="mv")
                            nc.vector.bn_aggr(out=mv[:], in_=st6[:, c, :, :])
                            nc.vector.tensor_scalar_mul(out=stats_loc[:, c, 0:1],
                                                        in0=mv[:, 0:1], scalar1=float(T))
                            m2c = stat.tile([128, 1], FP32, tag="m2c")
                            nc.vector.tensor_tensor(out=m2c[:], in0=mv[:, 0:1],
                                                    in1=mv[:, 0:1], op=AOT.mult)
                            nc.vector.tensor_tensor(out=m2c[:], in0=mv[:, 1:2],
                                                    in1=m2c[:], op=AOT.add)
                            nc.vector.tensor_scalar_mul(out=stats_loc[:, c, 1:2],
                                                        in0=m2c[:], scalar1=float(T))
                        # one batched all-reduce for all four chunks' stats
                        cc_in = dpool.tile([128, 8], FP32, tag="cci")
                        cc_out = dpool.tile([128, 8], FP32, tag="cco")
                        nc.gpsimd.dma_start(
                            cc_in[:], stats_loc[:].rearrange("p a b -> p (a b)"))
                        nc.gpsimd.collective_compute(
                            "AllReduce", AOT.add,
                            replica_groups=[list(range(N_CORES))],
                            ins=[cc_in.opt()], outs=[cc_out.opt()])
                        nc.gpsimd.dma_start(
                            gstats[:].rearrange("p a b -> p (a b)"), cc_out[:])
                        # PE warm-keeper bridging the reduce latency
                        with tc.tile_pool(name="wkp", bufs=1) as wkp, \
                             tc.tile_pool(name="pswk", bufs=1, space="PSUM") as pswk:
                            junk = wkp.tile([128, 512], FP32R, tag="junk")
                            nc.vector.tensor_scalar(
                                out=junk[:], in0=x[:, 0, 0:512].bitcast(FP32),
                                scalar1=stats_loc[:, NC - 1, 0:1], scalar2=None,
                                op0=AOT.mult)
                            junk_ps = pswk.tile([128, 512], FP32, tag="jps")
                            for _i in range(88):
                                nc.tensor.matmul(junk_ps[:], eye_r[:], junk[:],
                                                 start=True, stop=True)
                    sil = convpre.tile([128, NC, T], BF16, tag="cin")
                    # per-chunk BN math + normalize + SiLU
                    for c in range(NC):
                        gm = stat.tile([128, 1], FP32, tag=f"gm{c}", name=f"gm{c}")
                        sf = stat.tile([128, 1], FP32, tag=f"sf{c}", name=f"sf{c}")
                        nc.vector.tensor_scalar_mul(out=gm[:], in0=gstats[:, c, 0:1],
                                                    scalar1=1.0 / ntot)
                        nc.vector.tensor_tensor(out=sf[:], in0=gm[:], in1=gm[:],
                                                op=AOT.mult)
                        nc.vector.scalar_tensor_tensor(
                            out=sf[:], in0=gstats[:, c, 1:2], scalar=1.0 / ntot,
                            in1=sf[:], op0=AOT.mult, op1=AOT.subtract)
                        nc.scalar.activation(out=sf[:], in_=sf[:], func=AF.Sqrt,
                                             bias=eps_sb[:], scale=1.0)
                        nc.vector.reciprocal(out=sf[:], in_=sf[:])
                        nc.vector.tensor_tensor(out=sf[:], in0=sf[:],
                                                in1=bng_sb[:, c:c + 1], op=AOT.mult)
                        nc.vector.tensor_scalar(
                            out=conv_raw[:, c, :], in0=conv_raw[:, c, :],
                            scalar1=gm[:], scalar2=sf[:],
                            op0=AOT.subtract, op1=AOT.mult)
                        nc.scalar.activation(
                            out=sil[:, c, 0:T], in_=conv_raw[:, c, :],
                            func=AF.Silu, bias=bnb_sb[:, c:c + 1], scale=1.0)
                    # pointwise: kc-pair partials start as sil chunks arrive
                    with tc.tile_pool(name="wpp", bufs=1) as wpool, \
                         tc.tile_pool(name="psp", bufs=1, space="PSUM") as psum:
                        wpw_sb = wpool.tile([128, NC, D], BF16, tag="wpw")
                        nc.sync.dma_start(wpw_sb[:], chunked(wpwT_d))
                        pws = {}
                        for mc in range(NC):
                            for nh in range(2):
                                pws[(mc, nh)] = psum.tile([128, 512], FP32,
                                                          tag=f"pw{mc}{nh}",
                                                          name=f"pw{mc}{nh}")
                        for kc in range(NC):
                            for mc in range(NC):
                                for nh in range(2):
                                    nc.tensor.matmul(
                                        pws[(mc, nh)][:],
                                        wpw_sb[:, kc, 128 * mc:128 * (mc + 1)],
                                        sil[:, kc, 512 * nh:512 * (nh + 1)],
                                        start=(kc == 0), stop=(kc == NC - 1))
                        for mc in range(NC):
                            for nh in range(2):
                                nc.vector.tensor_tensor(
                                    out=x[:, mc, 512 * nh:512 * (nh + 1)],
                                    in0=pws[(mc, nh)][:],
                                    in1=x[:, mc, 512 * nh:512 * (nh + 1)].bitcast(FP32),
                                    op=AOT.add)

            # ================= FFN2 =================
            with nc.named_scope("ffn2"):
                z4 = layer_norm("4", zdt=FP8)
                ffn(z4, w2, "2")

            # ================= LN5 fused into transpose-out =================
            with nc.named_scope("ln5out"):
                if has_ln5gb:
                    z5 = layer_norm("5")
                    g5s = bias1.tile([128, 4], FP32, tag="g5")
                    b5s = bias1.tile([128, 4], FP32, tag="b5")
                    nc.sync.dma_start(g5s[:], g5_d.ap())
                    nc.sync.dma_start(b5s[:], b5_d.ap())
                    for c in range(NC):
                        nc.vector.tensor_scalar(
                            out=z5[:, c, :], in0=z5[:, c, :].bitcast(FP32),
                            scalar1=g5s[:, c:c + 1], scalar2=b5s[:, c:c + 1],
                            op0=AOT.mult, op1=AOT.add)
                    with tc.tile_pool(name="pst", bufs=4, space="PSUM") as psum, \
                         tc.tile_pool(name="outp", bufs=1) as outp:
                        out_sb = outp.tile([128, 8, NC, 128], FP32, tag="outsb")
                        for tt in range(8):
                            for c in range(NC):
                                tp = psum.tile([128, 128], FP32R, tag="tp")
                                nc.tensor.transpose(
                                    tp[:], z5[:, c, 128 * tt:128 * (tt + 1)], eye_r[:])
                                nc.scalar.copy(out=out_sb[:, tt, c, :],
                                               in_=tp[:].bitcast(FP32))
                            nc.sync.dma_start(
                                out_flat[128 * tt:128 * (tt + 1), :],
                                out_sb[:, tt, :, :].rearrange("p c f -> p (c f)"))
                else:
                    st5, r5 = layer_norm("5", apply=False)
                    with tc.tile_pool(name="pst", bufs=4, space="PSUM") as psum, \
                         tc.tile_pool(name="outp", bufs=1) as outp:
                        out_sb = outp.tile([128, 8, NC, 128], FP32, tag="outsb")
                        for tt in range(8):
                            for c in range(NC):
                                tp = psum.tile([128, 128], FP32R, tag="tp")
                                nc.tensor.transpose(
                                    tp[:], x[:, c, 128 * tt:128 * (tt + 1)], eye_r[:])
                                nc.vector.tensor_scalar(
                                    out=out_sb[:, tt, c, :],
                                    in0=tp[:].bitcast(FP32),
                                    scalar1=st5[:, tt, 0:1],
                                    scalar2=r5[:, tt:tt + 1].bitcast(FP32),
                                    op0=AOT.subtract, op1=AOT.mult)
                            nc.sync.dma_start(
                                out_flat[128 * tt:128 * (tt + 1), :],
                                out_sb[:, tt, :, :].rearrange("p c f -> p (c f)"))

    nc.compile()
    return nc


# ---------------------------------------------------------------- entry point

def kernel(**inputs):
    d, xs, flags = _prep_host(inputs)
    if flags not in _CACHE:
        _CACHE[flags] = _build(flags)
    nc = _CACHE[flags]
    in_maps = [dict(d, x_fm=xs[c]) for c in range(N_CORES)]
    res = run_bass_kernel_spmd(nc, in_maps, core_ids=list(range(N_CORES)))
    out = np.concatenate([res.results[c]["out"] for c in range(N_CORES)], axis=0)
    return np.ascontiguousarray(out.astype(np.float32))


def run_traced(**inputs):
    """test-only helper: returns (out, BassKernelResults-with-trace)."""
    import ntff_shim
    ntff_shim.install()
    d, xs, flags = _prep_host(inputs)
    if flags not in _CACHE:
        _CACHE[flags] = _build(flags)
    nc = _CACHE[flags]
    in_maps = [dict(d, x_fm=xs[c]) for c in range(N_CORES)]
    res = run_bass_kernel_spmd(nc, in_maps, core_ids=list(range(N_CORES)), trace=True)
    out = np.concatenate([res.results[c]["out"] for c in range(N_CORES)], axis=0)
    return np.ascontiguousarray(out.astype(np.float32)), res



# revision 82
# speedup vs baseline: 1.0387x; 1.0387x over previous
"""Trainium2 Bass kernel for nn_ConformerEncoderLayer (B=16, L=512, D=512, H=8, FF=2048).

Sharding: data-parallel over batch across 8 NeuronCores (2 batch elems / core).
Device layout: feature-major residual stream x_fm [D=512 (4x128 chunks), T=1024].

v4 (fp8): both FFN matmul pairs and the depthwise conv run fp8e4m3 with
DoubleRow (two K-tiles per instruction = 2x PE throughput; weights host-scaled
x1024 into the fp8 normal range, descaled via the ACT free affine / DVE
immediates). The conv input is duplicated into an even + one-column-shifted
odd copy so DoubleRow tap PAIRS read a standard strided [128, 2, 512] AP; the
per-channel fp8 scaling of the conv taps is absorbed exactly by BatchNorm.
The q/k projections also run fp8 DoubleRow (rope output hr stored fp8); GLU /
pointwise / scores / AV / out-proj stay bf16 (error budget: measured rel err
1.60e-2 vs the 2e-2 gate; numpy bit-accurate sim attributes per-path fp8 error).
Rope tables are bf16 (DVE 2x packed mode, half the DMA). BatchNorm stats use
one batched 8-core all-reduce with a PE junk-matmul keep-warm bridging its
latency; pointwise partials and per-chunk normalize pipeline into the tail.
Constant DMAs are queued before the 2MB x transfer so the PE warmup burst
(HAM un-throttle) is not blocked; E->O conv copies ride the gpsimd queue to
avoid head-of-line blocking the sync queue's weight prefetches.

v3: token-major LayerNorm stats (PE transpose + 128-lane math + stride-0
matmul broadcast); softmax normalization deferred to batch end (one ACT
reciprocal per batch, no ACT-table thrash in the head loop); weight pools
hoisted so FFN2/conv weight DMAs prefetch during earlier phases; PE warmup
burst for the HAM clock.
"""
import sys

sys.path.insert(0, "/opt/trn_rl_repo")

import contextlib
import numpy as np
import ml_dtypes
import concourse.bacc as bacc
import concourse.tile as tile
from concourse import mybir
from concourse.bass_utils import run_bass_kernel_spmd

FP32 = mybir.dt.float32
FP32R = mybir.dt.float32r
BF16 = mybir.dt.bfloat16
FP8 = mybir.dt.float8e4
NP8 = ml_dtypes.float8_e4m3
SCL = 1024.0          # host-side weight pre-scale (2^10) for fp8 range
ISCL = 1.0 / SCL
KTAP2 = 32            # taps padded to even count for DoubleRow pairs

N_CORES = 8
B, L, D, H, FF = 16, 512, 512, 8, 2048
HD = D // H                # 64
BL = B // N_CORES          # 2 batch elems per core
T = BL * L                 # 1024 tokens per core
NC = D // 128              # 4 feature chunks
ROPE_BASE = 10000.0
LN_EPS = 1e-5
BN_EPS = 1e-5
KTAP = 31
PAD = 15
CONVW = PAD + L + PAD + L + PAD          # zeros|b0|zeros|b1|zeros = 1069
OFF_B = (PAD, PAD + L + PAD)             # start col of each batch's data

_CACHE = {}


# ---------------------------------------------------------------- host prep

def _prep_host(inp):
    f32 = np.float32
    x = np.asarray(inp["x"], f32)
    ln = {k: np.asarray(inp[k], f32) for k in
          ("ln1_g", "ln1_b", "ln2_g", "ln2_b", "ln3_g", "ln3_b",
           "ln4_g", "ln4_b", "ln5_g", "ln5_b")}

    def colvec(b, n):  # [F] -> [128, F//128]  (tile[p, i] = b[i*128+p])
        return np.ascontiguousarray(np.asarray(b, f32).reshape(n, 128).T)

    w_ff1a = np.asarray(inp["w_ff1a"], f32)
    w_ff1b = np.asarray(inp["w_ff1b"], f32)
    w_ff2a = np.asarray(inp["w_ff2a"], f32)
    w_ff2b = np.asarray(inp["w_ff2b"], f32)

    def to8(w):  # pre-scaled fp8 weight (descaled on device via ISCL)
        return np.ascontiguousarray(np.clip(w * SCL, -224, 224).astype(NP8))

    d = {}
    ba1 = np.asarray(inp["b_ff1a"], f32) + ln["ln1_b"] @ w_ff1a.T
    ba2 = np.asarray(inp["b_ff2a"], f32) + ln["ln4_b"] @ w_ff2a.T
    d["wff1a"] = to8((w_ff1a * ln["ln1_g"][None, :]).T)
    d["bff1a"] = colvec(ba1, 16)
    d["wff1b"] = to8(0.5 * w_ff1b.T)
    d["bff1b"] = colvec(0.5 * np.asarray(inp["b_ff1b"], f32), 4)
    d["wff2a"] = to8((w_ff2a * ln["ln4_g"][None, :]).T)
    d["bff2a"] = colvec(ba2, 16)
    d["wff2b"] = to8(0.5 * w_ff2b.T)
    d["bff2b"] = colvec(0.5 * np.asarray(inp["b_ff2b"], f32), 4)
    has_bffa = bool(np.any(ba1) or np.any(ba2))
    has_bffb = bool(np.any(inp["b_ff1b"]) or np.any(inp["b_ff2b"]))

    w_in = np.asarray(inp["w_in"], f32)
    b_in = np.asarray(inp["b_in"], f32)
    wq, wk, wv = w_in[:D], w_in[D:2 * D], w_in[2 * D:]
    bq, bk, bv = b_in[:D], b_in[D:2 * D], b_in[2 * D:]
    d["wqT"] = to8(wq.T)
    d["wkT"] = to8(wk.T)
    d["wvT"] = np.ascontiguousarray(
        ((wv * ln["ln2_g"][None, :]).T).astype(ml_dtypes.bfloat16))
    d["bq"] = colvec(bq, 4)
    d["bk"] = colvec(bk, 4)
    d["bv_row"] = np.ascontiguousarray((bv + ln["ln2_b"] @ wv.T).reshape(1, D))
    w_out = np.asarray(inp["w_out"], f32)
    d["woutTh"] = to8(
        w_out.T.reshape(H, HD, D).transpose(1, 0, 2))          # [64, 8, 512]
    d["bout"] = colvec(np.asarray(inp["b_out"], f32), 4)
    has_bout = bool(np.any(inp["b_out"]))

    pos = np.arange(L, dtype=f32)
    num = np.arange(0, D, 2, dtype=f32) / D
    scale = (1.0 / ROPE_BASE ** num).astype(ml_dtypes.bfloat16).astype(f32)
    theta = pos[:, None] * scale[None, :]                      # [512, 256]
    cosT, sinT = np.cos(theta).T, np.sin(theta).T              # [256, 512]
    cos2 = np.concatenate([cosT] * BL, axis=1)                 # [256, 1024]
    sin2 = np.concatenate([sinT] * BL, axis=1)
    g1h = ln["ln2_g"][:D // 2][:, None]
    g2h = ln["ln2_g"][D // 2:][:, None]
    ropetab = np.stack(
        [g1h * cos2, g1h * sin2, g2h * cos2, g2h * sin2])      # [4, 256, T]
    # device layout [128, 4, 2, T] (partition-major, contiguous DMA); bf16
    # so the rope tensor_tensor chain runs in the DVE 2x packed mode
    d["ropetab"] = np.ascontiguousarray(
        ropetab.reshape(4, 2, 128, T).transpose(2, 0, 1, 3)
        .astype(ml_dtypes.bfloat16))

    has_qkfix = not np.allclose(ln["ln2_b"], 0.0)
    if has_qkfix:
        bb = np.broadcast_to(ln["ln2_b"], (L, D)).astype(f32)
        half = D // 2
        rb = np.concatenate([bb[:, :half] * cosT.T - bb[:, half:] * sinT.T,
                             bb[:, half:] * cosT.T + bb[:, :half] * sinT.T], axis=1)
        qfix = np.concatenate([(rb @ wq.T).T] * BL, axis=1)
        kfix = np.concatenate([(rb @ wk.T).T] * BL, axis=1)
        d["qkfix"] = np.ascontiguousarray(np.stack([qfix, kfix]))  # [2, 512, 1024]

    w_glu = np.asarray(inp["w_glu"], f32)
    # a-half scaled x16 so conv_in (fp8) sits higher above the subnormal
    # cutoff; BN downstream absorbs the uniform scale exactly.
    CINS = 16.0
    wglu_s = w_glu * ln["ln3_g"][None, :]
    wglu_s[:D] *= CINS
    d["wgluT"] = np.ascontiguousarray(wglu_s.T.astype(ml_dtypes.bfloat16))
    bglu_s = ln["ln3_b"] @ w_glu.T
    bglu_s[:D] *= CINS
    d["bglu"] = colvec(bglu_s, 8)
    w_dw = np.asarray(inp["w_dw"], f32)[:, 0, :]               # [512, 31]
    # per-channel power-of-2 scale into fp8 range; BN normalization
    # absorbs any per-channel scale on the conv output exactly.
    cmax = np.abs(w_dw).max(axis=1)
    cscl = np.exp2(np.floor(np.log2(128.0 / np.maximum(cmax, 1e-30))))
    w_dws = w_dw * cscl[:, None]
    diag = np.zeros((NC, 128, KTAP2, 128), f32)
    idx = np.arange(128)
    for c in range(NC):
        diag[c, idx, :KTAP, idx] = w_dws[128 * c + idx, :]
    d["diag"] = np.ascontiguousarray(np.clip(diag, -224, 224).astype(NP8))
    d["bng"] = colvec(np.asarray(inp["bn_g"], f32), 4)
    d["bnb"] = colvec(np.asarray(inp["bn_b"], f32), 4)
    d["wpwT"] = np.ascontiguousarray(
        np.asarray(inp["w_pw"], f32).T.astype(ml_dtypes.bfloat16))
    d["eye"] = np.eye(128, dtype=f32)
    d["cconst"] = np.full((128, 1), 1.0 / D, f32)
    d["rowones"] = np.ones((1, 128), f32)
    d["rowones_bf"] = np.ones((1, 128), ml_dtypes.bfloat16)
    d["padzero"] = np.zeros((128, NC, PAD), NP8)

    ln5_nontrivial = not (np.allclose(ln["ln5_g"], 1.0)
                          and np.allclose(ln["ln5_b"], 0.0))
    if ln5_nontrivial:
        d["g5"] = colvec(ln["ln5_g"], 4)
        d["b5"] = colvec(ln["ln5_b"], 4)

    xs = []
    for c in range(N_CORES):
        xc = x[BL * c: BL * (c + 1)]                           # [2, 512, 512]
        xs.append(np.ascontiguousarray(xc.transpose(2, 0, 1).reshape(D, T)))
    return d, xs, (has_qkfix, ln5_nontrivial, has_bffa, has_bffb, has_bout)


# ---------------------------------------------------------------- device build

def _build(flags):
    has_qkfix, has_ln5gb, has_bffa, has_bffb, has_bout = flags
    nc = bacc.Bacc("TRN2", target_bir_lowering=False, debug=False,
                   enable_asserts=True, num_devices=N_CORES)
    AOT = mybir.AluOpType
    AF = mybir.ActivationFunctionType
    DR = mybir.MatmulPerfMode.DoubleRow

    def din(name, shape, dt=FP32):
        return nc.dram_tensor(name, list(shape), dt, kind="ExternalInput")

    x_in = din("x_fm", [D, T])
    wff1a_d = din("wff1a", [D, FF], FP8); bff1a_d = din("bff1a", [128, 16])
    wff1b_d = din("wff1b", [FF, D], FP8); bff1b_d = din("bff1b", [128, 4])
    wff2a_d = din("wff2a", [D, FF], FP8); bff2a_d = din("bff2a", [128, 16])
    wff2b_d = din("wff2b", [FF, D], FP8); bff2b_d = din("bff2b", [128, 4])
    wqT_d = din("wqT", [D, D], FP8); wkT_d = din("wkT", [D, D], FP8)
    wvT_d = din("wvT", [D, D], BF16)
    bq_d = din("bq", [128, 4]); bk_d = din("bk", [128, 4]); bv_d = din("bv_row", [1, D])
    woutTh_d = din("woutTh", [HD, H, D], FP8); bout_d = din("bout", [128, 4])
    ropetab_d = din("ropetab", [128, 4, 2, T], BF16)
    wgluT_d = din("wgluT", [D, 2 * D], BF16); bglu_d = din("bglu", [128, 8])
    diag_d = din("diag", [NC, 128, KTAP2, 128], FP8)
    bng_d = din("bng", [128, 4]); bnb_d = din("bnb", [128, 4])
    wpwT_d = din("wpwT", [D, D], BF16)
    eye_d = din("eye", [128, 128])
    cconst_d = din("cconst", [128, 1])
    rowones_d = din("rowones", [1, 128])
    rowones_bf_d = din("rowones_bf", [1, 128], BF16)
    padzero_d = din("padzero", [128, NC, PAD], FP8)
    qkfix_d = din("qkfix", [2, D, T]) if has_qkfix else None
    g5_d = din("g5", [128, 4]) if has_ln5gb else None
    b5_d = din("b5", [128, 4]) if has_ln5gb else None
    out_d = nc.dram_tensor("out", [BL, L, D], FP32, kind="ExternalOutput")
    out_flat = out_d.ap().rearrange("b l d -> (b l) d")

    def chunked(ap_dram):
        return ap_dram.ap().rearrange("(c p) f -> p c f", p=128)

    with tile.TileContext(nc) as tc:
        ctx = contextlib.ExitStack()
        with ctx:
            resid = ctx.enter_context(tc.tile_pool(name="resid", bufs=1))
            zpool = ctx.enter_context(tc.tile_pool(name="zpool", bufs=1))
            scr = ctx.enter_context(tc.tile_pool(name="scr", bufs=1))
            sqp = ctx.enter_context(tc.tile_pool(name="sqp", bufs=2))
            stat = ctx.enter_context(tc.tile_pool(name="stat", bufs=1))
            bias1 = ctx.enter_context(tc.tile_pool(name="bias1", bufs=1))
            dpool = ctx.enter_context(tc.tile_pool(name="dpool", bufs=4, space="DRAM"))

            # ---------------- persistent tiles ----------------
            # small constants FIRST in the DMA queue: the PE warmup burst
            # depends only on eye_r, so it must not queue behind the 2MB x
            # transfer (that stalls the whole PE FIFO and cools the HAM).
            oneD_r = bias1.tile([128, 1], FP32R, tag="oneD_r")
            nc.sync.dma_start(oneD_r[:], cconst_d.ap().bitcast(FP32R))
            ones_row_r = bias1.tile([1, 128], FP32R, tag="ones_row")
            nc.sync.dma_start(ones_row_r[:], rowones_d.ap().bitcast(FP32R))
            ones_row64 = bias1.tile([HD + 1, 128], BF16, tag="ones_row64")
            nc.sync.dma_start(ones_row64[HD:HD + 1, :], rowones_bf_d.ap())
            eye_r = bias1.tile([128, 128], FP32R, tag="eye_r")
            nc.sync.dma_start(eye_r[:], eye_d.ap().bitcast(FP32R))
            eye_sb = bias1.tile([128, 128], FP32, tag="eye_sb")
            nc.sync.dma_start(eye_sb[:], eye_d.ap())
            x = resid.tile([128, NC, T], FP32R, tag="x")
            for _c in range(NC):
                nc.sync.dma_start(x[:, _c, :], chunked(x_in).bitcast(FP32R)[:, _c, :])
            eps_sb = bias1.tile([128, 1], FP32, tag="eps")
            nc.vector.memset(eps_sb[:], LN_EPS)
            rows_sb = bias1.tile([128, 1024], FP32, tag="rows")
            nc.vector.memset(rows_sb[:], 0.0)

            # ---------------- FFN1 weights (prefetch from t=0) ----------------
            wp1_cm = tc.tile_pool(name="wp1", bufs=1)
            wp1 = wp1_cm.__enter__()

            def load_ffn_w(wpool, wa_d, ba_d, wb_d, bb_d, tag):
                wa = wpool.tile([128, NC, FF], FP8, tag="wa" + tag)
                for _m in range(0, 16, 8):
                    nc.sync.dma_start(
                        wa[:, :, 128 * _m:128 * (_m + 8)],
                        chunked(wa_d)[:, :, 128 * _m:128 * (_m + 8)])
                wb = wpool.tile([128, 16, D], FP8, tag="wb" + tag)
                _wbap = wb_d.ap().rearrange("(c p) f -> p c f", p=128)
                for _k in range(0, 16, 8):
                    nc.sync.dma_start(wb[:, _k:_k + 8, :], _wbap[:, _k:_k + 8, :])
                ba = bias1.tile([128, 16], FP32, tag="ba" + tag)
                nc.sync.dma_start(ba[:], ba_d.ap())
                bb = bias1.tile([128, 4], FP32, tag="bb" + tag)
                nc.sync.dma_start(bb[:], bb_d.ap())
                return wa, wb, ba, bb

            w1 = load_ffn_w(wp1, wff1a_d, bff1a_d, wff1b_d, bff1b_d, "1")

            # ---------------- PE warmup (HAM K=8/8) ----------------
            # ~16 x 414ns fp32r matmuls = ~6.6us of PE busy, enough for one
            # HAM SHORT window; ends about when the x DMA lands so LN1
            # stats are not blocked behind it.
            with tc.tile_pool(name="pswm", bufs=1, space="PSUM") as pswm:
                wm_ps = pswm.tile([128, 128], FP32, tag="wm")
                for _i in range(16):
                    nc.tensor.matmul(wm_ps[:], eye_r[:], eye_r[:],
                                     start=True, stop=True)

            # ---------------- collective warm-up ----------------
            warm_sb = bias1.tile([128, 8], FP32, tag="warm")
            nc.vector.memset(warm_sb[:], 0.0)
            warm_in = dpool.tile([128, 8], FP32)
            warm_out = dpool.tile([128, 8], FP32)
            nc.gpsimd.dma_start(warm_in[:], warm_sb[:])
            nc.gpsimd.collective_compute(
                "AllReduce", AOT.add, replica_groups=[list(range(N_CORES))],
                ins=[warm_in.opt()], outs=[warm_out.opt()])
            nc.gpsimd.dma_start(warm_sb[:], warm_out[:])

            def act_reciprocal(out, in_):
                """ACT-engine reciprocal (bass guards it for accuracy; ~1e-5
                rel err measured on HW -- plenty for softmax denominators)."""
                bias = nc.scalar.bass.const_aps.scalar_like(0.0, in_)
                inputs = [nc.scalar.lower_ap(in_), nc.scalar.lower_ap(bias),
                          mybir.ImmediateValue(dtype=mybir.dt.float32, value=1.0),
                          mybir.ImmediateValue(dtype=mybir.dt.float32, value=0.0)]
                return nc.scalar.add_instruction(mybir.InstActivation(
                    name=nc.scalar.bass.get_next_instruction_name(),
                    func=AF.Reciprocal, ins=inputs,
                    outs=[nc.scalar.lower_ap(out)]))

            # ---------------- layer norm helper ----------------
            def layer_norm(tag, apply=True, zdt=BF16):
                """Standardize x over features.  Processed per token-half so
                the second half's stats overlap the first half's apply:
                row stats via matmul -> PE transpose -> 128-lane math ->
                stride-0 matmul broadcast -> 2 DVE ops per chunk-half."""
                sttag = "st5" if not apply else "st"
                rtag = "rt5" if not apply else "rt"
                st_sb = stat.tile([128, 8, 2], FP32, tag=sttag)
                r_tok = stat.tile([128, 8], FP32R, tag=rtag)
                mr_tok = None
                z = None
                tmp = None
                if apply:
                    mr_tok = stat.tile([128, 8], FP32R, tag="mrt")
                    z = zpool.tile([128, NC, T], zdt, tag="z")
                    tmp = scr.tile([128, T], FP32, tag="lntmp")
                with tc.tile_pool(name="psln" + tag, bufs=1, space="PSUM") as psln, \
                     tc.tile_pool(name="pstp" + tag, bufs=2, space="PSUM") as pstp, \
                     tc.tile_pool(name="psbc" + tag, bufs=1, space="PSUM") as psbc:
                    for nh in range(2):
                        sl = slice(512 * nh, 512 * (nh + 1))
                        mean_ps = psln.tile([1, 512], FP32, tag="mean")
                        msq_ps = psln.tile([1, 512], FP32, tag="msq")
                        for c in range(NC):
                            sq = sqp.tile([128, 512], FP32R, tag="sq")
                            nc.scalar.activation(
                                out=sq[:], in_=x[:, c, sl].bitcast(FP32),
                                func=AF.Square)
                            nc.tensor.matmul(mean_ps[:], oneD_r[:], x[:, c, sl],
                                             start=(c == 0), stop=(c == NC - 1))
                            nc.tensor.matmul(msq_ps[:], oneD_r[:], sq[:],
                                             start=(c == 0), stop=(c == NC - 1))
                        nc.vector.tensor_copy(rows_sb[0:1, sl], mean_ps[:])
                        nc.vector.tensor_copy(rows_sb[32:33, sl], msq_ps[:])
                        hb = slice(4 * nh, 4 * (nh + 1))
                        for tb in range(4 * nh, 4 * (nh + 1)):
                            st_ps = pstp.tile([128, 128], FP32, tag="stp")
                            nc.tensor.transpose(st_ps[:],
                                                rows_sb[:, 128 * tb:128 * (tb + 1)],
                                                eye_sb[:])
                            nc.vector.tensor_copy(
                                st_sb[:, tb, :],
                                st_ps[:, 0:64].rearrange(
                                    "p (a b) -> p a b", a=2)[:, :, 0])
                        m2 = stat.tile([128, 4], FP32, tag="m2t")
                        nc.scalar.activation(out=m2[:], in_=st_sb[:, hb, 0],
                                             func=AF.Square)
                        sig = stat.tile([128, 4], FP32, tag="sigt")
                        nc.vector.tensor_tensor(out=sig[:], in0=st_sb[:, hb, 1],
                                                in1=m2[:], op=AOT.subtract)
                        nc.scalar.activation(out=sig[:], in_=sig[:], func=AF.Sqrt,
                                             bias=eps_sb[:], scale=1.0)
                        with nc.allow_low_precision(reason="fp32r is rounded fp32"):
                            nc.vector.reciprocal(out=r_tok[:, hb], in_=sig[:])
                        if not apply:
                            continue
                        nc.vector.tensor_tensor(out=mr_tok[:, hb],
                                                in0=st_sb[:, hb, 0],
                                                in1=r_tok[:, hb].bitcast(FP32),
                                                op=AOT.mult)
                        bc_r = psbc.tile([128, 512], FP32, tag="bcr")
                        bc_mr = psbc.tile([128, 512], FP32, tag="bcmr")
                        for tb in range(4):
                            nc.tensor.matmul(
                                bc_r[:, 128 * tb:128 * (tb + 1)],
                                r_tok[:, 4 * nh + tb:4 * nh + tb + 1]
                                .to_broadcast((128, 128)),
                                eye_r[:], start=True, stop=True)
                            nc.tensor.matmul(
                                bc_mr[:, 128 * tb:128 * (tb + 1)],
                                mr_tok[:, 4 * nh + tb:4 * nh + tb + 1]
                                .to_broadcast((128, 128)),
                                eye_r[:], start=True, stop=True)
                        for c in range(NC):
                            nc.vector.tensor_tensor(
                                out=tmp[:, sl], in0=x[:, c, sl].bitcast(FP32),
                                in1=bc_r[:], op=AOT.mult)
                            nc.vector.tensor_tensor(
                                out=z[:, c, sl], in0=tmp[:, sl],
                                in1=bc_mr[:], op=AOT.subtract)
                if not apply:
                    return st_sb, r_tok
                return z

            def ffn(z, w, tag):
                wa, wb, ba, bb = w
                with tc.tile_pool(name="hp" + tag, bufs=1) as hpool, \
                     tc.tile_pool(name="psf" + tag, bufs=2, space="PSUM") as psum:
                    for nh in range(2):
                        hid = hpool.tile([128, 16, 512], FP8, tag="hid")
                        for mt in range(16):
                            p = psum.tile([128, 512], FP32, tag="mm")
                            for kc in range(0, NC, 2):
                                nc.tensor.matmul(
                                    p[:], wa[:, kc:kc + 2, 128 * mt:128 * (mt + 1)],
                                    z[:, kc:kc + 2, 512 * nh:512 * (nh + 1)],
                                    start=(kc == 0), stop=(kc == NC - 2),
                                    perf_mode=DR)
                            nc.scalar.activation(
                                out=hid[:, mt, :], in_=p[:],
                                func=AF.Relu, bias=ba[:, mt:mt + 1], scale=ISCL)
                        for mc in range(NC):
                            p = psum.tile([128, 512], FP32, tag="mm")
                            for kt in range(0, 16, 2):
                                nc.tensor.matmul(
                                    p[:], wb[:, kt:kt + 2, 128 * mc:128 * (mc + 1)],
                                    hid[:, kt:kt + 2, :],
                                    start=(kt == 0), stop=(kt == 14),
                                    perf_mode=DR)
                            if has_bffb:
                                tmp_b = hpool.tile([128, 512], FP32, tag="tb")
                                nc.vector.tensor_scalar(
                                    out=tmp_b[:], in0=p[:], scalar1=ISCL,
                                    scalar2=bb[:, mc:mc + 1],
                                    op0=AOT.mult, op1=AOT.add)
                                nc.vector.tensor_tensor(
                                    out=x[:, mc, 512 * nh:512 * (nh + 1)],
                                    in0=tmp_b[:],
                                    in1=x[:, mc, 512 * nh:512 * (nh + 1)].bitcast(FP32),
                                    op=AOT.add)
                            else:
                                nc.vector.scalar_tensor_tensor(
                                    out=x[:, mc, 512 * nh:512 * (nh + 1)],
                                    in0=p[:], scalar=ISCL,
                                    in1=x[:, mc, 512 * nh:512 * (nh + 1)].bitcast(FP32),
                                    op0=AOT.mult, op1=AOT.add)

            # ================= FFN1 =================
            with nc.named_scope("ffn1"):
                z1 = layer_norm("1", zdt=FP8)
                ffn(z1, w1, "1")
            wp1_cm.__exit__(None, None, None)

            # ---- conv_in + GLU weights: reserve + prefetch before attention ----
            # conv_in[:, 0] = GLU output (E copy); conv_in[:, 1] = same data
            # shifted left one column (O copy) so DoubleRow tap pairs
            # (2p, 2p+1) read [128, 2, 512] with a standard strided AP.
            convpre = ctx.enter_context(tc.tile_pool(name="convpre", bufs=1))
            conv_in = convpre.tile([128, 2, NC, CONVW], FP8, tag="cin")
            pz = padzero_d.ap()
            nc.sync.dma_start(conv_in[:, 0, :, 0:PAD], pz)
            nc.sync.dma_start(conv_in[:, 0, :, PAD + L:2 * PAD + L], pz)
            nc.sync.dma_start(conv_in[:, 0, :, 2 * PAD + 2 * L:CONVW], pz)
            nc.sync.dma_start(conv_in[:, 1, :, CONVW - 1:CONVW], pz[:, :, 0:1])
            wg_sb = convpre.tile([128, NC, 2 * D], BF16, tag="wg")
            nc.sync.dma_start(wg_sb[:], chunked(wgluT_d))
            bg_sb = bias1.tile([128, 8], FP32, tag="bg")
            nc.sync.dma_start(bg_sb[:], bglu_d.ap())

            # ================= attention =================
            with nc.named_scope("attn"):
                z2 = layer_norm("2")
                with tc.tile_pool(name="apool", bufs=1) as apool:
                    # bf16 q / zero-padded k (K=128 keeps the PE array fully
                    # active for scores; the zero half kills the other head)
                    q_sb = apool.tile([128, NC, T], BF16, tag="q")
                    k_ev = apool.tile([128, NC, T], BF16, tag="kev")
                    k_od = apool.tile([128, NC, T], BF16, tag="kod")
                    nc.vector.tensor_scalar(
                        out=k_ev[64:128, :, :], in0=x[64:128, :, :].bitcast(FP32),
                        scalar1=0.0, scalar2=0.0, op0=AOT.mult, op1=AOT.add)
                    nc.vector.tensor_scalar(
                        out=k_od[0:64, :, :], in0=x[0:64, :, :].bitcast(FP32),
                        scalar1=0.0, scalar2=0.0, op0=AOT.mult, op1=AOT.add)
                    v_aug = apool.tile([128, 8, H, HD + 2], FP8, tag="vaug")
                    nc.vector.tensor_scalar(
                        out=v_aug[:, :, :, HD:HD + 1],
                        in0=eye_sb[:, 0:64].rearrange(
                            "p (a b) -> p a b", a=8)[:, :, :, None],
                        scalar1=0.0, scalar2=1.0, op0=AOT.mult, op1=AOT.add)
                    with tc.tile_pool(name="wvp", bufs=1) as wvp, \
                         tc.tile_pool(name="psa", bufs=3, space="PSUM") as psum:
                        # ---- v projection (token-major, ones-augmented) ----
                        wv_sb = wvp.tile([128, NC, D], BF16, tag="wv")
                        nc.sync.dma_start(wv_sb[:], chunked(wvT_d))
                        bvr = bias1.tile([1, D], FP32R, tag="bvr")
                        nc.sync.dma_start(bvr[:], bv_d.ap().bitcast(FP32R))
                        for tt in range(8):
                            p = psum.tile([128, 512], FP32, tag="mm")
                            for kc in range(NC):
                                nc.tensor.matmul(p[:], z2[:, kc, 128 * tt:128 * (tt + 1)],
                                                 wv_sb[:, kc, :],
                                                 start=(kc == 0), stop=False)
                            nc.tensor.matmul(p[:], ones_row_r[:], bvr[:],
                                             start=False, stop=True)
                            nc.scalar.copy(out=v_aug[:, tt, :, 0:HD],
                                           in_=p[:].rearrange("p (h f) -> p h f", h=H))
                        # PE keep-warm while the rope DVE chain runs
                        with tc.tile_pool(name="pswa", bufs=1, space="PSUM") as pswa:
                            junk_a = pswa.tile([128, 512], FP32, tag="jpa")
                            for _i in range(24):
                                nc.tensor.matmul(junk_a[:], eye_r[:],
                                                 x[:, 0, 0:512],
                                                 start=True, stop=True)
                        # ---- rope + q/k ----
                        if has_qkfix:
                            qkf = apool.tile([128, 2, NC, T], FP32, tag="qkf")
                            nc.sync.dma_start(
                                qkf[:],
                                qkfix_d.ap().rearrange("k (c p) t -> p k c t", p=128))
                        with tc.tile_pool(name="hrp", bufs=1) as hrp:
                            hr = hrp.tile([128, NC, T], FP8, tag="hr")
                            with tc.tile_pool(name="tabp", bufs=1) as tabp:
                                tab = tabp.tile([128, 4, 2, T], BF16, tag="ropetab")
                                nc.sync.dma_start(tab[:], ropetab_d.ap())
                                rtmp = tabp.tile([128, T], BF16, tag="rtmp")
                                rtmp2 = tabp.tile([128, T], BF16, tag="rtmp2")
                                for c in range(2):
                                    nc.vector.tensor_tensor(
                                        out=rtmp[:], in0=z2[:, c, :],
                                        in1=tab[:, 0, c, :], op=AOT.mult)
                                    nc.vector.tensor_tensor(
                                        out=rtmp2[:], in0=z2[:, c + 2, :],
                                        in1=tab[:, 3, c, :], op=AOT.mult)
                                    nc.vector.tensor_tensor(
                                        out=hr[:, c, :], in0=rtmp[:], in1=rtmp2[:],
                                        op=AOT.subtract)
                                    nc.vector.tensor_tensor(
                                        out=rtmp[:], in0=z2[:, c + 2, :],
                                        in1=tab[:, 2, c, :], op=AOT.mult)
                                    nc.vector.tensor_tensor(
                                        out=rtmp2[:], in0=z2[:, c, :],
                                        in1=tab[:, 1, c, :], op=AOT.mult)
                                    nc.vector.tensor_tensor(
                                        out=hr[:, c + 2, :], in0=rtmp[:], in1=rtmp2[:],
                                        op=AOT.add)
                            with tc.tile_pool(name="wqkp", bufs=1) as wqkp:
                                wq_sb = wqkp.tile([128, NC, D], FP8, tag="wq")
                                wk_sb = wqkp.tile([128, NC, D], FP8, tag="wk")
                                nc.sync.dma_start(wq_sb[:], chunked(wqT_d))
                                nc.sync.dma_start(wk_sb[:], chunked(wkT_d))
                                bqs = bias1.tile([128, 4], FP32, tag="bqs")
                                bks = bias1.tile([128, 4], FP32, tag="bks")
                                nc.sync.dma_start(bqs[:], bq_d.ap())
                                nc.sync.dma_start(bks[:], bk_d.ap())
                                qk_tmp = None
                                if has_qkfix:
                                    qk_tmp = wqkp.tile([128, 512], FP32, tag="qkt")
                                for mt in range(NC):
                                    for nh in range(2):
                                        p = psum.tile([128, 512], FP32, tag="mm")
                                        for kc in range(0, NC, 2):
                                            nc.tensor.matmul(
                                                p[:], wq_sb[:, kc:kc + 2,
                                                            128 * mt:128 * (mt + 1)],
                                                hr[:, kc:kc + 2,
                                                   512 * nh:512 * (nh + 1)],
                                                start=(kc == 0), stop=(kc == NC - 2),
                                                perf_mode=DR)
                                        if has_qkfix:
                                            nc.vector.tensor_scalar(
                                                out=qk_tmp[:], in0=p[:],
                                                scalar1=ISCL,
                                                scalar2=bqs[:, mt:mt + 1],
                                                op0=AOT.mult, op1=AOT.add)
                                            nc.vector.tensor_tensor(
                                                out=q_sb[:, mt, 512 * nh:512 * (nh + 1)],
                                                in0=qk_tmp[:],
                                                in1=qkf[:, 0, mt,
                                                        512 * nh:512 * (nh + 1)],
                                                op=AOT.add)
                                        else:
                                            nc.vector.tensor_scalar(
                                                out=q_sb[:, mt, 512 * nh:512 * (nh + 1)],
                                                in0=p[:], scalar1=ISCL,
                                                scalar2=bqs[:, mt:mt + 1],
                                                op0=AOT.mult, op1=AOT.add)
                                for mt in range(NC):
                                    for nh in range(2):
                                        p = psum.tile([128, 512], FP32, tag="mm")
                                        for kc in range(0, NC, 2):
                                            nc.tensor.matmul(
                                                p[:], wk_sb[:, kc:kc + 2,
                                                            128 * mt:128 * (mt + 1)],
                                                hr[:, kc:kc + 2,
                                                   512 * nh:512 * (nh + 1)],
                                                start=(kc == 0), stop=(kc == NC - 2),
                                                perf_mode=DR)
                                        if has_qkfix:
                                            nc.vector.tensor_scalar(
                                                out=qk_tmp[:], in0=p[:],
                                                scalar1=ISCL,
                                                scalar2=bks[:, mt:mt + 1],
                                                op0=AOT.mult, op1=AOT.add)
                                            nc.vector.tensor_tensor(
                                                out=k_ev[0:64, mt, 512 * nh:512 * (nh + 1)],
                                                in0=qk_tmp[0:64, :],
                                                in1=qkf[0:64, 1, mt,
                                                        512 * nh:512 * (nh + 1)],
                                                op=AOT.add)
                                            nc.vector.tensor_tensor(
                                                out=k_od[64:128, mt, 512 * nh:512 * (nh + 1)],
                                                in0=qk_tmp[64:128, :],
                                                in1=qkf[64:128, 1, mt,
                                                        512 * nh:512 * (nh + 1)],
                                                op=AOT.add)
                                        else:
                                            nc.vector.tensor_scalar(
                                                out=k_ev[0:64, mt, 512 * nh:512 * (nh + 1)],
                                                in0=p[0:64, :], scalar1=ISCL,
                                                scalar2=bks[0:64, mt:mt + 1],
                                                op0=AOT.mult, op1=AOT.add)
                                            nc.vector.tensor_scalar(
                                                out=k_od[64:128, mt, 512 * nh:512 * (nh + 1)],
                                                in0=p[64:128, :], scalar1=ISCL,
                                                scalar2=bks[64:128, mt:mt + 1],
                                                op0=AOT.mult, op1=AOT.add)
                    # ---- scores / AV: software-pipelined, one Exp per head ----
                    with tc.tile_pool(name="wop", bufs=1) as wop, \
                         tc.tile_pool(name="osbp", bufs=2) as osbp:
                        wo_sb = wop.tile([HD, H, D], FP8, tag="wo")
                        nc.sync.dma_start(wo_sb[:], woutTh_d.ap())
                        bo_sb = bias1.tile([128, 4], FP32, tag="bo")
                        nc.sync.dma_start(bo_sb[:], bout_d.ap())
                        o_raws = []
                        o_norms = []
                        for b in range(BL):
                            o_raw = osbp.tile([HD + 2, H, 512], BF16, tag="osb")
                            o_raws.append(o_raw)
                            o_norm = osbp.tile([HD, H, 512], FP8, tag="onrm",
                                               name=f"onrm{b}")
                            o_norms.append(o_norm)
                        iters = [(h, b) for h in range(H) for b in range(BL)]
                        SKEW = 1
                        ets = {}
                        with tc.tile_pool(name="pss", bufs=3, space="PSUM") as pss, \
                             tc.tile_pool(name="psav", bufs=2, space="PSUM") as psav, \
                             tc.tile_pool(name="ep", bufs=3) as epool:
                            for i in range(len(iters) + SKEW):
                                if i < len(iters):
                                    h, b = iters[i]
                                    kp = k_ev if h % 2 == 0 else k_od
                                    ch = h // 2
                                    e_t = epool.tile([128, 4, 512], FP8, tag="e")
                                    for half in range(2):
                                        s_ps = pss.tile([128, 2, 512], FP32, tag="sps")
                                        for kk in range(2):
                                            kt = 2 * half + kk
                                            nc.tensor.matmul(
                                                s_ps[:, kk, :],
                                                kp[:, ch, 512 * b + 128 * kt:
                                                   512 * b + 128 * (kt + 1)],
                                                q_sb[:, ch, 512 * b:512 * (b + 1)],
                                                start=True, stop=True)
                                        nc.scalar.activation(
                                            out=e_t[:, 2 * half:2 * half + 2, :]
                                            .rearrange("p a b -> p (a b)"),
                                            in_=s_ps[:].rearrange("p a b -> p (a b)"),
                                            func=AF.Exp, scale=1.0 / 8.0)
                                    ets[i] = e_t
                                if i >= SKEW:
                                    h, b = iters[i - SKEW]
                                    e_t = ets.pop(i - SKEW)
                                    o_ps = psav.tile([HD + 2, 512], FP32, tag="avo")
                                    for kt in range(0, 4, 2):
                                        nc.tensor.matmul(
                                            o_ps[:],
                                            v_aug[:, 4 * b + kt:4 * b + kt + 2, h, :],
                                            e_t[:, kt:kt + 2, :],
                                            start=(kt == 0), stop=(kt == 2),
                                            perf_mode=DR)
                                    with nc.allow_low_precision(reason="fp8 attn"):
                                        nc.vector.tensor_copy(o_raws[b][:, h, :], o_ps[:])
                        # ---- normalization + out-proj ----
                        with tc.tile_pool(name="psbch", bufs=2, space="PSUM") as psbch, \
                             tc.tile_pool(name="pso", bufs=3, space="PSUM") as pso:
                            for b in range(BL):
                                act_reciprocal(
                                    o_raws[b][HD:HD + 1, :, :].rearrange(
                                        "p h t -> p (h t)"),
                                    o_raws[b][HD:HD + 1, :, :].rearrange(
                                        "p h t -> p (h t)"))
                                for h in range(H):
                                    bch = psbch.tile([HD, 512], FP32, tag="bch")
                                    nc.tensor.matmul(bch[:],
                                                     ones_row64[HD:HD + 1, 0:HD],
                                                     o_raws[b][HD:HD + 1, h, :],
                                                     start=True, stop=True)
                                    with nc.allow_low_precision(reason="fp8 attn"):
                                        nc.vector.tensor_tensor(
                                            out=o_norms[b][:, h, :],
                                            in0=o_raws[b][0:HD, h, :],
                                            in1=bch[:], op=AOT.mult)
                                for mc in range(NC):
                                    p = pso.tile([128, 512], FP32, tag="mm")
                                    for h in range(0, H, 2):
                                        nc.tensor.matmul(
                                            p[:], wo_sb[:, h:h + 2,
                                                        128 * mc:128 * (mc + 1)],
                                            o_norms[b][:, h:h + 2, :],
                                            start=(h == 0), stop=(h == H - 2),
                                            perf_mode=DR)
                                    if has_bout:
                                        o_tmp = wop.tile([128, 512], FP32, tag="otmp")
                                        nc.vector.tensor_scalar(
                                            out=o_tmp[:], in0=p[:], scalar1=ISCL,
                                            scalar2=bo_sb[:, mc:mc + 1],
                                            op0=AOT.mult, op1=AOT.add)
                                        nc.vector.tensor_tensor(
                                            out=x[:, mc, 512 * b:512 * (b + 1)],
                                            in0=o_tmp[:],
                                            in1=x[:, mc, 512 * b:512 * (b + 1)].bitcast(FP32),
                                            op=AOT.add)
                                    else:
                                        nc.vector.scalar_tensor_tensor(
                                            out=x[:, mc, 512 * b:512 * (b + 1)],
                                            in0=p[:], scalar=ISCL,
                                            in1=x[:, mc, 512 * b:512 * (b + 1)].bitcast(FP32),
                                            op0=AOT.mult, op1=AOT.add)

            # ---- FFN2 weights: reserve + prefetch during conv ----
            wp2 = ctx.enter_context(tc.tile_pool(name="wp2", bufs=1))
            w2 = load_ffn_w(wp2, wff2a_d, bff2a_d, wff2b_d, bff2b_d, "2")

            # ================= conv module =================
            with nc.named_scope("conv"):
                z3 = layer_norm("3")
                bng_sb = bias1.tile([128, 4], FP32, tag="bngw")
                bnb_sb = bias1.tile([128, 4], FP32, tag="bnbw")
                nc.sync.dma_start(bng_sb[:], bng_d.ap())
                nc.sync.dma_start(bnb_sb[:], bnb_d.ap())
                ntot = float(N_CORES * T)
                with tc.tile_pool(name="cpool", bufs=1) as cpool:
                    conv_raw = cpool.tile([128, NC, T], BF16, tag="craw")
                    st6 = stat.tile([128, NC, 2, 6], FP32, tag="st6")
                    stats_loc = stat.tile([128, NC, 2], FP32, tag="bnloc")
                    gstats = stat.tile([128, NC, 2], FP32, tag="bngl")
                    # per-chunk: GLU -> E/O copy -> dw conv -> local stats;
                    # one batched all-reduce after the last chunk
                    with tc.tile_pool(name="sigp", bufs=2) as sigp, \
                         tc.tile_pool(name="diagp", bufs=2) as diagp, \
                         tc.tile_pool(name="psg", bufs=2, space="PSUM") as psg, \
                         tc.tile_pool(name="psc", bufs=2, space="PSUM") as psc:
                        for c in range(NC):
                            for nh in range(2):
                                p_a = psg.tile([128, 512], FP32, tag="pa")
                                p_s = psg.tile([128, 512], FP32, tag="psg")
                                for kc in range(NC):
                                    nc.tensor.matmul(
                                        p_s[:], wg_sb[:, kc,
                                                      128 * (c + 4):128 * (c + 5)],
                                        z3[:, kc, 512 * nh:512 * (nh + 1)],
                                        start=(kc == 0), stop=(kc == NC - 1))
                                for kc in range(NC):
                                    nc.tensor.matmul(
                                        p_a[:], wg_sb[:, kc, 128 * c:128 * (c + 1)],
                                        z3[:, kc, 512 * nh:512 * (nh + 1)],
                                        start=(kc == 0), stop=(kc == NC - 1))
                                sig = sigp.tile([128, 512], FP32, tag="sig")
                                nc.scalar.activation(out=sig[:], in_=p_s[:],
                                                     func=AF.Sigmoid,
                                                     bias=bg_sb[:, c + 4:c + 5],
                                                     scale=1.0)
                                nc.vector.scalar_tensor_tensor(
                                    out=conv_in[:, 0, c, OFF_B[nh]:OFF_B[nh] + L],
                                    in0=p_a[:], scalar=bg_sb[:, c:c + 1],
                                    in1=sig[:], op0=AOT.add, op1=AOT.mult)
                            # O copy = E shifted one column left (pads carry
                            # over); on the gpsimd queue so its wait on the
                            # GLU STT doesn't head-of-line-block the sync
                            # queue's weight prefetches
                            nc.gpsimd.dma_start(conv_in[:, 1, c, 0:CONVW - 1],
                                                conv_in[:, 0, c, 1:CONVW])
                            # dw conv: 16 DoubleRow tap-pairs per (chunk, batch)
                            diag = diagp.tile([128, KTAP2, 128], FP8, tag="diag")
                            nc.sync.dma_start(diag[:], diag_d.ap()[c])
                            cps = []
                            for _b in range(BL):
                                cpb = psc.tile([128, 512], FP32, tag="cps")
                                cps.append(cpb)
                            for pp in range(KTAP2 // 2):
                                for b in range(BL):
                                    s0 = OFF_B[b] - PAD + 2 * pp
                                    nc.tensor.matmul(
                                        cps[b][:], diag[:, 2 * pp:2 * pp + 2, :],
                                        conv_in[:, :, c, s0:s0 + L],
                                        start=(pp == 0), stop=(pp == KTAP2 // 2 - 1),
                                        perf_mode=DR)
                            for b in range(BL):
                                nc.vector.bn_stats(out=st6[:, c, b, :], in_=cps[b][:])
                                nc.scalar.copy(out=conv_raw[:, c, L * b:L * (b + 1)],
                                               in_=cps[b][:])
                            # local stats (scaled to sums)
                            mv = stat.tile([128, 2], FP32, tag="mv")
                            nc.vector.bn_aggr(out=mv[:], in_=st6[:, c, :, :])
                            nc.vector.tensor_scalar_mul(out=stats_loc[:, c, 0:1],
                                                        in0=mv[:, 0:1], scalar1=float(T))
                            m2c = stat.tile([128, 1], FP32, tag="m2c")
                            nc.vector.tensor_tensor(out=m2c[:], in0=mv[:, 0:1],
                                                    in1=mv[:, 0:1], op=AOT.mult)
                            nc.vector.tensor_tensor(out=m2c[:], in0=mv[:, 1:2],
                                                    in1=m2c[:], op=AOT.add)
                            nc.vector.tensor_scalar_mul(out=stats_loc[:, c, 1:2],
                                                        in0=m2c[:], scalar1=float(T))
                        # one batched all-reduce for all four chunks' stats
                        cc_in = dpool.tile([128, 8], FP32, tag="cci")
                        cc_out = dpool.tile([128, 8], FP32, tag="cco")
                        nc.gpsimd.dma_start(
                            cc_in[:], stats_loc[:].rearrange("p a b -> p (a b)"))
                        nc.gpsimd.collective_compute(
                            "AllReduce", AOT.add,
                            replica_groups=[list(range(N_CORES))],
                            ins=[cc_in.opt()], outs=[cc_out.opt()])
                        nc.gpsimd.dma_start(
                            gstats[:].rearrange("p a b -> p (a b)"), cc_out[:])
                        # PE warm-keeper bridging the reduce latency
                        with tc.tile_pool(name="wkp", bufs=1) as wkp, \
                             tc.tile_pool(name="pswk", bufs=1, space="PSUM") as pswk:
                            junk = wkp.tile([128, 512], FP32R, tag="junk")
                            nc.vector.tensor_scalar(
                                out=junk[:], in0=x[:, 0, 0:512].bitcast(FP32),
                                scalar1=stats_loc[:, NC - 1, 0:1], scalar2=None,
                                op0=AOT.mult)
                            junk_ps = pswk.tile([128, 512], FP32, tag="jps")
                            for _i in range(160):
                                nc.tensor.matmul(junk_ps[:], eye_r[:], junk[:],
                                                 start=True, stop=True)
                    sil = convpre.tile([128, NC, T], BF16, tag="cin")
                    # per-chunk BN math + normalize + SiLU
                    for c in range(NC):
                        gm = stat.tile([128, 1], FP32, tag=f"gm{c}", name=f"gm{c}")
                        sf = stat.tile([128, 1], FP32, tag=f"sf{c}", name=f"sf{c}")
                        nc.vector.tensor_scalar_mul(out=gm[:], in0=gstats[:, c, 0:1],
                                                    scalar1=1.0 / ntot)
                        nc.vector.tensor_tensor(out=sf[:], in0=gm[:], in1=gm[:],
                                                op=AOT.mult)
                        nc.vector.scalar_tensor_tensor(
                            out=sf[:], in0=gstats[:, c, 1:2], scalar=1.0 / ntot,
                            in1=sf[:], op0=AOT.mult, op1=AOT.subtract)
                        nc.scalar.activation(out=sf[:], in_=sf[:], func=AF.Sqrt,
                                             bias=eps_sb[:], scale=1.0)
                        nc.vector.reciprocal(out=sf[:], in_=sf[:])
                        nc.vector.tensor_tensor(out=sf[:], in0=sf[:],
                                                in1=bng_sb[:, c:c + 1], op=AOT.mult)
                        nc.vector.tensor_scalar(
                            out=conv_raw[:, c, :], in0=conv_raw[:, c, :],
                            scalar1=gm[:], scalar2=sf[:],
                            op0=AOT.subtract, op1=AOT.mult)
                        nc.scalar.activation(
                            out=sil[:, c, 0:T], in_=conv_raw[:, c, :],
                            func=AF.Silu, bias=bnb_sb[:, c:c + 1], scale=1.0)
                    # pointwise: kc-pair partials start as sil chunks arrive
                    with tc.tile_pool(name="wpp", bufs=1) as wpool, \
                         tc.tile_pool(name="psp", bufs=1, space="PSUM") as psum:
                        wpw_sb = wpool.tile([128, NC, D], BF16, tag="wpw")
                        nc.sync.dma_start(wpw_sb[:], chunked(wpwT_d))
                        pws = {}
                        for mc in range(NC):
                            for nh in range(2):
                                pws[(mc, nh)] = psum.tile([128, 512], FP32,
                                                          tag=f"pw{mc}{nh}",
                                                          name=f"pw{mc}{nh}")
                        for kc in range(NC):
                            for mc in range(NC):
                                for nh in range(2):
                                    nc.tensor.matmul(
                                        pws[(mc, nh)][:],
                                        wpw_sb[:, kc, 128 * mc:128 * (mc + 1)],
                                        sil[:, kc, 512 * nh:512 * (nh + 1)],
                                        start=(kc == 0), stop=(kc == NC - 1))
                        for mc in range(NC):
                            for nh in range(2):
                                nc.vector.tensor_tensor(
                                    out=x[:, mc, 512 * nh:512 * (nh + 1)],
                                    in0=pws[(mc, nh)][:],
                                    in1=x[:, mc, 512 * nh:512 * (nh + 1)].bitcast(FP32),
                                    op=AOT.add)

            # ================= FFN2 =================
            with nc.named_scope("ffn2"):
                z4 = layer_norm("4", zdt=FP8)
                ffn(z4, w2, "2")

            # ================= LN5 fused into transpose-out =================
            with nc.named_scope("ln5out"):
                if has_ln5gb:
                    z5 = layer_norm("5")
                    g5s = bias1.tile([128, 4], FP32, tag="g5")
                    b5s = bias1.tile([128, 4], FP32, tag="b5")
                    nc.sync.dma_start(g5s[:], g5_d.ap())
                    nc.sync.dma_start(b5s[:], b5_d.ap())
                    for c in range(NC):
                        nc.vector.tensor_scalar(
                            out=z5[:, c, :], in0=z5[:, c, :].bitcast(FP32),
                            scalar1=g5s[:, c:c + 1], scalar2=b5s[:, c:c + 1],
                            op0=AOT.mult, op1=AOT.add)
                    with tc.tile_pool(name="pst", bufs=4, space="PSUM") as psum, \
                         tc.tile_pool(name="outp", bufs=1) as outp:
                        out_sb = outp.tile([128, 8, NC, 128], FP32, tag="outsb")
                        for tt in range(8):
                            for c in range(NC):
                                tp = psum.tile([128, 128], FP32R, tag="tp")
                                nc.tensor.transpose(
                                    tp[:], z5[:, c, 128 * tt:128 * (tt + 1)], eye_r[:])
                                nc.scalar.copy(out=out_sb[:, tt, c, :],
                                               in_=tp[:].bitcast(FP32))
                            nc.sync.dma_start(
                                out_flat[128 * tt:128 * (tt + 1), :],
                                out_sb[:, tt, :, :].rearrange("p c f -> p (c f)"))
                else:
                    st5, r5 = layer_norm("5", apply=False)
                    with tc.tile_pool(name="pst", bufs=4, space="PSUM") as psum, \
                         tc.tile_pool(name="outp", bufs=1) as outp:
                        out_sb = outp.tile([128, 8, NC, 128], FP32, tag="outsb")
                        for tt in range(8):
                            for c in range(NC):
                                tp = psum.tile([128, 128], FP32R, tag="tp")
                                nc.tensor.transpose(
                                    tp[:], x[:, c, 128 * tt:128 * (tt + 1)], eye_r[:])
                                nc.vector.tensor_scalar(
                                    out=out_sb[:, tt, c, :],
                                    in0=tp[:].bitcast(FP32),
                                    scalar1=st5[:, tt, 0:1],
                                    scalar2=r5[:, tt:tt + 1].bitcast(FP32),
                                    op0=AOT.subtract, op1=AOT.mult)
                            nc.sync.dma_start(
                                out_flat[128 * tt:128 * (tt + 1), :],
                                out_sb[:, tt, :, :].rearrange("p c f -> p (c f)"))

    nc.compile()
    return nc


# ---------------------------------------------------------------- entry point

def kernel(**inputs):
    d, xs, flags = _prep_host(inputs)
    if flags not in _CACHE:
        _CACHE[flags] = _build(flags)
    nc = _CACHE[flags]
    in_maps = [dict(d, x_fm=xs[c]) for c in range(N_CORES)]
    res = run_bass_kernel_spmd(nc, in_maps, core_ids=list(range(N_CORES)))
    out = np.concatenate([res.results[c]["out"] for c in range(N_CORES)], axis=0)
    return np.ascontiguousarray(out.astype(np.float32))


def run_traced(**inputs):
    """test-only helper: returns (out, BassKernelResults-with-trace)."""
    import ntff_shim
    ntff_shim.install()
    d, xs, flags = _prep_host(inputs)
    if flags not in _CACHE:
        _CACHE[flags] = _build(flags)
    nc = _CACHE[flags]
    in_maps = [dict(d, x_fm=xs[c]) for c in range(N_CORES)]
    res = run_bass_kernel_spmd(nc, in_maps, core_ids=list(range(N_CORES)), trace=True)
    out = np.concatenate([res.results[c]["out"] for c in range(N_CORES)], axis=0)
    return np.ascontiguousarray(out.astype(np.float32)), res

